# revision 55
# baseline (speedup 1.0000x reference)
import sys, os
sys.path.insert(0, '/opt/trn_rl_repo')
import numpy as np
import ml_dtypes

import concourse.bass as bass
import concourse.bacc as bacc
import concourse.mybir as mybir
import concourse.tile as tile
from concourse.bass_utils import run_bass_kernel_spmd

F32 = mybir.dt.float32
I32 = mybir.dt.int32
BF = mybir.dt.bfloat16
AF = mybir.ActivationFunctionType
OP = mybir.AluOpType
AX = mybir.AxisListType
SCALE = 12.0
BF_NP = ml_dtypes.bfloat16


class Cfg:
    def __init__(self, V=50000, D=128, B=1024, P=50, NC=8, PADP=64):
        assert D == 128
        self.V, self.D, self.B, self.P, self.NC, self.PADP = V, D, B, P, NC, PADP
        self.SC = B // NC                    # sessions per core
        assert 128 % PADP == 0 and P <= PADP
        self.SPT = 128 // PADP               # sessions per node-tile
        self.NT = self.SC * PADP // 128      # node tiles per core
        assert self.SC == 128                # one session-tile per core


FULL = Cfg()


def build_nc(cfg, dt_val, has_t0, n_cores):
    c = cfg
    NT, SPT, PADP, V = c.NT, c.SPT, c.PADP, c.V
    SCH = 8    # m12t stream chunk (node tiles per dma)
    CH = 16    # stage-tail chunk (tiles)
    TSC = 2048                       # tgt stream columns per dma
    NSTR = (V + TSC - 1) // TSC      # tgt stream steps
    nc = bacc.Bacc("TRN2", target_bir_lowering=False, debug=False, num_devices=n_cores)

    def din(name, shape, dtype=BF):
        return nc.dram_tensor(name, shape, dtype, kind="ExternalInput")

    x0 = din("x0", [128, NT, 128])
    x0T = din("x0T", [128, NT, 128])
    m12tT = din("m12tT", [128, NT, 256])
    sthT = din("sthT", [128, NT, 128])
    stfT = din("stfT", [128, NT, 128])
    st0T = din("st0T", [128, NT, 128]) if has_t0 else None
    tgtT = din("tgtT", [128, V])
    w_p1 = din("w_p1", [128, 384])
    w_p2 = din("w_p2", [128, 384])
    w_whhT = din("w_whhT", [128, 384])
    w_xrz = din("w_xrz", [128, 256])
    w_xh = din("w_xh", [128, 128])
    w_hrz = din("w_hrz", [128, 256])
    w_hh = din("w_hh", [128, 128])
    w_fcu = din("w_fcu", [128, 128])
    w_fcvw = din("w_fcvw", [128, 128])
    w_fsra = din("w_fsra", [128, 128])
    w_fsrb = din("w_fsrb", [128, 128])
    b_pgx = din("b_pgx", [1, 512])   # [b_pg(384) | b_h3(128)]
    bpg_rep = din("bpg_rep", [128, 512])
    brz_rep = din("brz_rep", [128, 512])
    b_rz2 = din("b_rz2", [1, 512])   # [b_rz | b_rz]
    b_u2 = din("b_u2", [1, 256])     # [b_u | b_u]
    bvb_row = din("bvb_row", [1, 128])
    ones1 = din("ones1", [1, 128])
    ptf = din("ptf", [128, SPT])
    pt2 = din("pt2", [SPT, 128])
    sel = din("sel", [128, NT, 128])
    fce_rep = din("fce_rep", [128, 128])
    omz0_rep = din("omz0_rep", [128, 128])
    u0_rep = din("u0_rep", [128, 128])
    identity = din("identity", [128, 128])

    out_slice = nc.dram_tensor("out_slice", [c.SC, V], BF, kind="ExternalOutput")

    dt2 = float(dt_val) * 0.5
    dt6 = float(dt_val) / 6.0
    # NOTE: a single accumulating matmul spanning two separately-started PSUM
    # accumulation groups produced wrong results on HW; bias ones-MMs stay
    # per accumulation group.

    with tile.TileContext(nc) as tc, \
         nc.allow_low_precision("bf16 norm/exp partial sums fine for 2e-2 gate"):
        with tc.tile_pool(name="per", bufs=1) as per, \
             tc.tile_pool(name="str", bufs=2) as strm, \
             tc.tile_pool(name="sc", bufs=3) as sc, \
             tc.tile_pool(name="scd", bufs=5) as scD, \
             tc.tile_pool(name="ob", bufs=4) as ob, \
             tc.tile_pool(name="pse", bufs=2, space="PSUM") as psE, \
             tc.tile_pool(name="psu", bufs=3, space="PSUM") as psU, \
             tc.tile_pool(name="psa", bufs=2, space="PSUM") as psA2, \
             tc.tile_pool(name="psg", bufs=1, space="PSUM") as psG:
            state = tc.alloc_tile_pool(name="state", bufs=1)
            X = state.tile([128, NT, 128], BF, tag="X")
            H = state.tile([128, NT, 128], BF, tag="H")
            KS = state.tile([128, NT, 128], BF, tag="KS")
            DH = state.tile([128, NT, 128], BF, tag="DH")
            SQ = state.tile([128, 16, 128], BF, tag="SQ")  # norm_chunk scratch (CH=16)
            STH = state.tile([128, NT, 128], BF, tag="STH")
            STF = state.tile([128, NT, 128], BF, tag="STF")

            def ld(t, shape, dtype=BF):
                s = per.tile(shape, dtype, tag="c_" + t.name)
                nc.sync.dma_start(out=s[:], in_=t[:])
                return s

            p1_s = ld(w_p1, [128, 384]); p2_s = ld(w_p2, [128, 384])
            whhT_s = ld(w_whhT, [128, 384])
            xrz_s = ld(w_xrz, [128, 256]); xh_s = ld(w_xh, [128, 128])
            hrz_s = ld(w_hrz, [128, 256]); hh_s = ld(w_hh, [128, 128])
            fcu_s = ld(w_fcu, [128, 128]); fcvw_s = ld(w_fcvw, [128, 128])
            fsra_s = ld(w_fsra, [128, 128]); fsrb_s = ld(w_fsrb, [128, 128])
            bpgx_s = ld(b_pgx, [1, 512])
            bpgr_s = ld(bpg_rep, [128, 512])
            brzr_s = ld(brz_rep, [128, 512])
            brz2_s = ld(b_rz2, [1, 512])
            bu2_s = ld(b_u2, [1, 256])
            bvb_s = ld(bvb_row, [1, 128])
            ones_s = ld(ones1, [1, 128])
            ptf_s = ld(ptf, [128, SPT]); pt2_s = ld(pt2, [SPT, 128])
            sel_s = state.tile([128, NT, 128], BF, tag="SEL")
            nc.sync.dma_start(out=sel_s[:], in_=sel[:])
            fce_s = ld(fce_rep, [128, 128])
            id_s = ld(identity, [128, 128])
            omz0_s = u0_s = None
            if not has_t0:
                omz0_s = ld(omz0_rep, [128, 128])
                u0_s = ld(u0_rep, [128, 128])

            # state load first; St matrices issued after the GGNN stream so the
            # m12 chunks aren't queued behind 4MB of eval-time data
            nc.sync.dma_start(out=X[:], in_=x0[:])
            # alias onto KS: KS is first written in the first stage_tail,
            # after the GGNN loop (XT0's last reader) completes
            XT0 = state.tile([128, NT, 128], BF, tag="KS")
            nc.sync.dma_start(out=XT0[:], in_=x0T[:])
            ST0 = None
            if has_t0:
                ST0 = state.tile([128, NT, 128], BF, tag="ST0")

            MM = nc.tensor.matmul

            # ================= GGNN layer =================
            # z-columns of P1/P2/whhT/b_pg are host-negated, so one sigmoid
            # over pg[0:256] yields [r | 1-z]. Software-pipelined one step:
            # agg matmuls a step ahead of the gate chain.
            GNS = NT // 2
            gnp = [None] * GNS; gn12 = [None] * GNS; gpg = [None] * GNS

            def gA(s):  # PE: weighted-mean aggregation
                j = 2 * s
                if s % (SCH // 2) == 0:
                    mt = strm.tile([128, SCH, 256], BF, tag="bigstream", name="mt")
                    jn = min(SCH, NT - j)
                    nc.sync.dma_start(out=mt[:, :jn, :], in_=m12tT[:, j:j + jn, :])
                    gA.mt = mt
                mt = gA.mt
                jj = j % SCH
                a = psA2.tile([128, 512], F32, tag="aggP", space="PSUM", name="nP")
                MM(out=a[:, 0:256], lhsT=X[:, j, :], rhs=mt[:, jj, :],
                   start=True, stop=True, skip_group_check=True)
                MM(out=a[:, 256:512], lhsT=X[:, j + 1, :], rhs=mt[:, jj + 1, :],
                   start=True, stop=True, skip_group_check=True)
                gnp[s] = a

            def gB(s):  # ACT: PSUM -> SBUF
                gn12[s] = sc.tile([128, 512], BF, tag="n12s", name="n12")
                nc.scalar.copy(out=gn12[s][:], in_=gnp[s][:])
                gnp[s] = None

            def gC(s):  # PE: gate matmuls (k=0 bias on PE, k=1 via DVE in gD)
                j = 2 * s
                n12 = gn12[s]
                pgs = []
                for k in range(2):
                    o = 256 * k
                    pool = psE if k == 0 else psU
                    tag = "przP" if k == 0 else "puP"
                    pg = pool.tile([128, 512], F32, tag=tag, space="PSUM", name="pg")
                    MM(out=pg[:, 0:384], lhsT=n12[:, o:o + 128], rhs=p1_s[:],
                       start=True, stop=False, skip_group_check=True)
                    MM(out=pg[:, 0:256], lhsT=XT0[:, j + k, :],
                       rhs=whhT_s[:, 0:256], start=False, stop=False, skip_group_check=True)
                    if k == 0:
                        MM(out=pg[:, 0:384], lhsT=n12[:, o + 128:o + 256], rhs=p2_s[:],
                           start=False, stop=False, skip_group_check=True)
                        MM(out=pg[:, 0:384], lhsT=ones_s[:], rhs=bpgx_s[:, 0:384],
                           start=False, stop=True, skip_group_check=True)
                        MM(out=pg[:, 384:512], lhsT=XT0[:, j + k, :],
                           rhs=whhT_s[:, 256:384], start=True, stop=False, skip_group_check=True)
                        MM(out=pg[:, 384:512], lhsT=ones_s[:], rhs=bpgx_s[:, 384:512],
                           start=False, stop=True, skip_group_check=True)
                    else:
                        MM(out=pg[:, 0:384], lhsT=n12[:, o + 128:o + 256], rhs=p2_s[:],
                           start=False, stop=True, skip_group_check=True)
                        MM(out=pg[:, 384:512], lhsT=XT0[:, j + k, :],
                           rhs=whhT_s[:, 256:384], start=True, stop=True, skip_group_check=True)
                    pgs.append(pg)
                gpg[s] = pgs
                gn12[s] = None

            def gD(s):  # gates + state update
                j = 2 * s
                sigP = sc.tile([128, 2, 256], BF, tag="gsig", name="sigP")
                ntP = sc.tile([128, 2, 128], BF, tag="gnt", name="ntP")
                t2 = sc.tile([128, 2, 128], BF, tag="t1", name="t2")
                for k in range(2):
                    pg = gpg[s][k]
                    if k == 1:
                        nc.vector.tensor_tensor(out=pg[:], in0=pg[:], in1=bpgr_s[:],
                                                op=OP.add)
                    nc.scalar.activation(out=sigP[:, k, :], in_=pg[:, 0:256], func=AF.Sigmoid)
                    nc.vector.tensor_tensor(out=t2[:, k, :], in0=sigP[:, k, 0:128],
                                            in1=pg[:, 384:512], op=OP.mult)
                    nc.vector.tensor_tensor(out=t2[:, k, :], in0=t2[:, k, :],
                                            in1=pg[:, 256:384], op=OP.add)
                nc.scalar.activation(out=ntP[:], in_=t2[:], func=AF.Tanh)
                nc.gpsimd.tensor_tensor(out=ntP[:], in0=ntP[:], in1=X[:, j:j + 2, :],
                                        op=OP.subtract)
                nc.gpsimd.tensor_tensor(out=ntP[:], in0=ntP[:], in1=sigP[:, :, 128:256],
                                        op=OP.mult)
                nc.vector.tensor_tensor(out=X[:, j:j + 2, :], in0=X[:, j:j + 2, :],
                                        in1=ntP[:], op=OP.add)
                gpg[s] = None

            for s in range(GNS + 1):
                if s < GNS:
                    gA(s)
                if s >= 1:
                    gC(s - 1)
                if s < GNS:
                    gB(s)
                if s >= 1:
                    gD(s - 1)

            # eval-time data: issued after the GGNN stream DMAs
            nc.sync.dma_start(out=STH[:], in_=sthT[:])
            nc.sync.dma_start(out=STF[:], in_=stfT[:])
            if has_t0:
                nc.sync.dma_start(out=ST0[:], in_=st0T[:])

            def rsqrt_dve(n2, W, cmul, outdt, tag):
                """cs = cmul * n2**-0.5 on DVE only (bit trick + 2 Newton iters;
                ~5e-6 rel err). Avoids ACT Sqrt table thrash vs sigmoid/tanh."""
                ii = sc.tile([128, W], I32, tag=tag + "_i", name="ii")
                nc.vector.tensor_scalar(out=ii[:], in0=n2[:].bitcast(I32), scalar1=1,
                                        scalar2=None, op0=OP.logical_shift_right)
                nc.vector.tensor_scalar(out=ii[:], in0=ii[:], scalar1=0x5f3759df,
                                        scalar2=-1, op0=OP.subtract, op1=OP.mult)
                y0 = ii[:].bitcast(F32)
                t1 = sc.tile([128, W], F32, tag=tag + "_t", name="t1")
                y = sc.tile([128, W], F32, tag=tag + "_y", name="y")
                nc.vector.tensor_tensor(out=t1[:], in0=y0, in1=y0, op=OP.mult)
                nc.vector.tensor_tensor(out=t1[:], in0=t1[:], in1=n2[:], op=OP.mult)
                nc.vector.tensor_scalar(out=t1[:], in0=t1[:], scalar1=-0.5, scalar2=1.5,
                                        op0=OP.mult, op1=OP.add)
                nc.vector.tensor_tensor(out=y[:], in0=t1[:], in1=y0, op=OP.mult)
                nc.vector.tensor_tensor(out=t1[:], in0=y[:], in1=y[:], op=OP.mult)
                nc.vector.tensor_tensor(out=t1[:], in0=t1[:], in1=n2[:], op=OP.mult)
                nc.vector.tensor_scalar(out=t1[:], in0=t1[:],
                                        scalar1=-0.5 * cmul, scalar2=1.5 * cmul,
                                        op0=OP.mult, op1=OP.add)
                cs = sc.tile([128, W], outdt, tag=tag + "_c", name="cs")
                nc.vector.tensor_tensor(out=cs[:], in0=t1[:], in1=y[:], op=OP.mult)
                return cs

            def norm_chunk(arr, c0, eps, cmul):
                """cs[:, c0:c0+CH] = cmul / max(|row|, eps) for one CH-tile chunk.
                All-DVE: squares+reduce then bit-trick rsqrt."""
                n2 = sc.tile([128, CH], F32, tag="nrm_n2", name="n2")
                nc.vector.tensor_tensor(out=SQ[:, :CH, :], in0=arr[:, c0:c0 + CH, :],
                                        in1=arr[:, c0:c0 + CH, :], op=OP.mult)
                nc.vector.tensor_reduce(out=n2[:], in_=SQ[:, :CH, :],
                                        axis=AX.X, op=OP.add)
                nc.vector.tensor_scalar_max(out=n2[:], in0=n2[:], scalar1=max(eps * eps, 1e-37))
                return rsqrt_dve(n2, CH, cmul, BF, "nrm")

            # ================= ODE: RK4 =================
            first_ks = [True]

            def tail_chunks(c_stage, rho, last):
                f = float(rho) / float(c_stage)
                first = first_ks[0]
                first_ks[0] = False

                def mk(c0):
                    def run():
                        c1 = c0 + CH
                        cs = norm_chunk(DH, c0, 1e-12, c_stage)
                        nc.vector.tensor_tensor(
                            out=DH[:, c0:c1, :], in0=DH[:, c0:c1, :],
                            in1=cs[:, :, None].to_broadcast([128, CH, 128]),
                            op=OP.mult)
                        if not last:
                            nc.gpsimd.tensor_tensor(out=H[:, c0:c1, :],
                                                    in0=X[:, c0:c1, :],
                                                    in1=DH[:, c0:c1, :], op=OP.add)
                        if first:
                            nc.vector.tensor_scalar_mul(out=KS[:, c0:c1, :],
                                                        in0=DH[:, c0:c1, :], scalar1=f)
                        else:
                            nc.vector.scalar_tensor_tensor(
                                out=KS[:, c0:c1, :], in0=DH[:, c0:c1, :], scalar=f,
                                in1=KS[:, c0:c1, :], op0=OP.mult, op1=OP.add)
                    return run
                return [mk(c0) for c0 in range(0, NT, CH)]

            def first_stage_chunks():
                """X normalize + t=0 shortcut dh + first tail, per chunk."""
                tl = tail_chunks(dt2, dt6, False)

                def mk(c0):
                    def run():
                        c1 = c0 + CH
                        cs = norm_chunk(X, c0, 1e-12, 1.0)
                        nc.vector.tensor_tensor(
                            out=X[:, c0:c1, :], in0=X[:, c0:c1, :],
                            in1=cs[:, :, None].to_broadcast([128, CH, 128]),
                            op=OP.mult)
                        nc.vector.tensor_tensor(
                            out=DH[:, c0:c1, :],
                            in0=u0_s[:, None, :].to_broadcast([128, CH, 128]),
                            in1=X[:, c0:c1, :], op=OP.subtract)
                        nc.gpsimd.tensor_tensor(
                            out=DH[:, c0:c1, :], in0=DH[:, c0:c1, :],
                            in1=omz0_s[:, None, :].to_broadcast([128, CH, 128]),
                            op=OP.mult)
                        tl[c0 // CH]()
                    return run
                return [mk(c0) for c0 in range(0, NT, CH)]

            def full_eval(st_res, pre):
                # z-columns of xrz/hrz/b_rz host-negated -> sigmoid gives [r | 1-z].
                # Software-pipelined: stage deps are >=1 step old so each
                # engine queue never head-of-line blocks.
                NS = NT // 2
                agg = [None] * NS; prz = [None] * NS; pu = [None] * NS
                sxt = [None] * NS; ghT = [None] * NS; sig = [None] * NS
                rh = [None] * NS; uT = [None] * NS

                def stA(s):  # PE: aggregation matmuls
                    j = 2 * s
                    a = psA2.tile([128, 512], F32, tag="aggP", space="PSUM")
                    MM(out=a[:, 0:128], lhsT=X[:, j, :], rhs=st_res[:, j, :],
                       start=True, stop=True, skip_group_check=True)
                    MM(out=a[:, 128:256], lhsT=X[:, j + 1, :], rhs=st_res[:, j + 1, :],
                       start=True, stop=True, skip_group_check=True)
                    MM(out=a[:, 256:384], lhsT=H[:, j, :], rhs=st_res[:, j, :],
                       start=True, stop=True, skip_group_check=True)
                    MM(out=a[:, 384:512], lhsT=H[:, j + 1, :], rhs=st_res[:, j + 1, :],
                       start=True, stop=True, skip_group_check=True)
                    agg[s] = a

                def stB(s):  # ACT+DVE: PSUM -> SBUF gate inputs
                    sxt[s] = scD.tile([128, 256], BF, tag="sxt", name="sxt")
                    nc.scalar.copy(out=sxt[s][:], in_=agg[s][:, 0:256])
                    ghT[s] = sc.tile([128, 256], BF, tag="ghT", name="ghT")
                    nc.vector.tensor_copy(out=ghT[s][:], in_=agg[s][:, 256:512])

                def stC(s):  # PE: r/z gate matmuls
                    p = psE.tile([128, 512], F32, tag="przP", space="PSUM")
                    for k in range(2):
                        o = 256 * k
                        MM(out=p[:, o:o + 256], lhsT=ghT[s][:, 128 * k:128 * (k + 1)],
                           rhs=hrz_s[:], start=True, stop=False, skip_group_check=True)
                        MM(out=p[:, o:o + 256], lhsT=sxt[s][:, 128 * k:128 * (k + 1)],
                           rhs=xrz_s[:], start=False, stop=True, skip_group_check=True)
                    prz[s] = p
                    ghT[s] = None

                def stD(s):  # DVE bias + ACT sigmoid
                    nc.vector.tensor_tensor(out=prz[s][:], in0=prz[s][:], in1=brzr_s[:],
                                            op=OP.add)
                    sig[s] = scD.tile([128, 4, 128], BF, tag="sig", name="sig")
                    nc.scalar.activation(out=sig[s][:], in_=prz[s][:], func=AF.Sigmoid)
                    prz[s] = None

                def stE(s):  # DVE: r * h
                    j = 2 * s
                    rh[s] = sc.tile([128, 2, 128], BF, tag="rh", name="rh")
                    nc.vector.tensor_tensor(out=rh[s][:], in0=sig[s][:, 0::2, :],
                                            in1=H[:, j:j + 2, :], op=OP.mult)

                def stF(s):  # PE: (r*h) aggregation matmuls
                    j = 2 * s
                    p = psU.tile([128, 512], F32, tag="puP", space="PSUM")
                    MM(out=p[:, 0:128], lhsT=rh[s][:, 0, :], rhs=st_res[:, j, :],
                       start=True, stop=True, skip_group_check=True)
                    MM(out=p[:, 128:256], lhsT=rh[s][:, 1, :], rhs=st_res[:, j + 1, :],
                       start=True, stop=True, skip_group_check=True)
                    pu[s] = p
                    rh[s] = None

                def stG(s):  # DVE: PSUM -> SBUF for u-gate lhsT
                    uT[s] = sc.tile([128, 256], BF, tag="uT", name="uT")
                    nc.vector.tensor_copy(out=uT[s][:], in_=pu[s][:, 0:256])

                def stH(s):  # PE: u gate matmuls
                    for k in range(2):
                        o = 256 + 128 * k
                        MM(out=pu[s][:, o:o + 128], lhsT=uT[s][:, 128 * k:128 * (k + 1)],
                           rhs=hh_s[:], start=True, stop=False, skip_group_check=True)
                        MM(out=pu[s][:, o:o + 128], lhsT=sxt[s][:, 128 * k:128 * (k + 1)],
                           rhs=xh_s[:], start=False, stop=False, skip_group_check=True)
                        MM(out=pu[s][:, o:o + 128], lhsT=ones_s[:], rhs=bu2_s[:, 0:128],
                           start=False, stop=True, skip_group_check=True)
                    uT[s] = None
                    sxt[s] = None

                def stI(s):  # ACT tanh + Pool epilogue -> DH
                    j = 2 * s
                    uP = sc.tile([128, 2, 128], BF, tag="ut")
                    nc.scalar.activation(out=uP[:], in_=pu[s][:, 256:512], func=AF.Tanh)
                    nc.gpsimd.tensor_tensor(out=uP[:], in0=uP[:], in1=H[:, j:j + 2, :],
                                            op=OP.subtract)
                    nc.gpsimd.tensor_tensor(out=DH[:, j:j + 2, :], in0=uP[:],
                                            in1=sig[s][:, 1::2, :], op=OP.mult)
                    pu[s] = None
                    sig[s] = None
                    agg[s] = None

                for s in range(NS + 3):
                    if s % (CH // 2) == 0 and s // (CH // 2) < len(pre):
                        pre[s // (CH // 2)]()
                    if s < NS:
                        stA(s)
                    if s >= 1 and s - 1 < NS:
                        stC(s - 1)
                    if s >= 2 and s - 2 < NS:
                        stF(s - 2)
                    if s >= 3 and s - 3 < NS:
                        stH(s - 3)
                    if s < NS:
                        stB(s)
                    if s >= 1 and s - 1 < NS:
                        stD(s - 1)
                        stE(s - 1)
                    if s >= 2 and s - 2 < NS:
                        stG(s - 2)
                    if s >= 3 and s - 3 < NS:
                        stI(s - 3)

            if has_t0:
                # ST0 path needs H initialized to X (f(0, feat) uses h=feat)
                def init_chunks():
                    def mk(c0):
                        def run():
                            c1 = c0 + CH
                            cs = norm_chunk(X, c0, 1e-12, 1.0)
                            nc.vector.tensor_tensor(
                                out=X[:, c0:c1, :], in0=X[:, c0:c1, :],
                                in1=cs[:, :, None].to_broadcast([128, CH, 128]),
                                op=OP.mult)
                            nc.vector.tensor_copy(out=H[:, c0:c1, :], in_=X[:, c0:c1, :])
                        return run
                    return [mk(c0) for c0 in range(0, NT, CH)]
                full_eval(ST0, init_chunks())
                full_eval(STH, tail_chunks(dt2, dt6, False))
            else:
                full_eval(STH, first_stage_chunks())
            full_eval(STH, tail_chunks(dt2, 2.0 * dt6, False))
            full_eval(STF, tail_chunks(float(dt_val), 2.0 * dt6, False))
            # final tail + H = normalize(X + KS), interleaved with readout
            # transposes chunk by chunk
            final_tail = tail_chunks(1.0, dt6, True)
            XT = state.tile([128, NT, 128], BF, tag="X")  # X dead after H formed
            flT = per.tile([128, 128], BF, tag="flTs")
            for c0 in range(0, NT, CH):
                c1 = c0 + CH
                final_tail[c0 // CH]()
                nc.vector.tensor_tensor(out=H[:, c0:c1, :], in0=X[:, c0:c1, :],
                                        in1=KS[:, c0:c1, :], op=OP.add)
                cs = norm_chunk(H, c0, 1e-30, 1.0)
                nc.vector.tensor_tensor(out=H[:, c0:c1, :], in0=H[:, c0:c1, :],
                                        in1=cs[:, :, None].to_broadcast([128, CH, 128]),
                                        op=OP.mult)
                for j in range(c0, c1, 2):
                    xtP = psU.tile([128, 256], BF, tag="puP", space="PSUM")
                    nc.tensor.transpose(out=xtP[:, 0:128], in_=H[:, j, :], identity=id_s[:])
                    nc.tensor.transpose(out=xtP[:, 128:256], in_=H[:, j + 1, :], identity=id_s[:])
                    if (j // 2) % 2 == 0:
                        nc.scalar.copy(out=XT[:, j:j + 2, :], in_=xtP[:])
                    else:
                        nc.vector.tensor_copy(out=XT[:, j:j + 2, :], in_=xtP[:])
                    nc.vector.tensor_copy(out=flT[:, j * SPT:(j + 2) * SPT],
                                          in_=XT[:, j:j + 2, c.P - 1::PADP])
            # fvS[sess, d] = H_last @ fc_vw + fc_vb   (sessions on partitions)
            pfv = psA2.tile([128, 512], F32, tag="aggP", space="PSUM")
            nc.tensor.matmul(out=pfv[:, 0:128], lhsT=flT[:], rhs=fcvw_s[:],
                             start=True, stop=False, skip_group_check=True)
            nc.tensor.matmul(out=pfv[:, 0:128], lhsT=ones_s[:], rhs=bvb_s[:],
                             start=False, stop=True, skip_group_check=True)
            fvS = per.tile([128, 128], BF, tag="fvS")
            nc.scalar.copy(out=fvS[:], in_=pfv[:, 0:128])

            ee = per.tile([128, NT], BF, tag="ee")
            ecolF = per.tile([128, NT], F32, tag="ecolF")
            for j in range(0, NT, 2):
                peP = psA2.tile([128, 512], F32, tag="aggP", space="PSUM")
                for k in range(2):
                    o = 128 * k
                    s0 = (j + k) * SPT
                    MM(out=peP[:, o:o + 128], lhsT=XT[:, j + k, :], rhs=fcu_s[:],
                       start=True, stop=False, skip_group_check=True)
                    # selection matmul: one-hot [sess, node] map adds fv[sess(n), :]
                    MM(out=peP[:, o:o + 128], lhsT=sel_s[:, j + k, :], rhs=fvS[:],
                       start=False, stop=True, skip_group_check=True)
                sg = sc.tile([128, 2, 128], BF, tag="sg")
                nc.scalar.activation(out=sg[:], in_=peP[:, 0:256], func=AF.Sigmoid)
                nc.vector.tensor_tensor(out=sg[:], in0=sg[:],
                                        in1=fce_s[:, None, :].to_broadcast([128, 2, 128]),
                                        op=OP.mult)
                nc.vector.tensor_reduce(out=ecolF[:, j:j + 2], in_=sg[:], axis=AX.X, op=OP.add)
            nc.scalar.activation(out=ee[:], in_=ecolF[:], func=AF.Exp)
            ssum_ps = psE.tile([SPT, NT], F32, tag="przP", space="PSUM")
            nc.tensor.matmul(out=ssum_ps[:], lhsT=ptf_s[:], rhs=ee[:], start=True, stop=True)
            rsum = per.tile([SPT, NT], F32, tag="rsum")
            nc.vector.reciprocal(out=rsum[:], in_=ssum_ps[:])
            rsumb = per.tile([SPT, NT], BF, tag="rsumb")
            nc.vector.tensor_copy(out=rsumb[:], in_=rsum[:])
            sb_ps = psE.tile([128, NT], F32, tag="przP", space="PSUM")
            nc.tensor.matmul(out=sb_ps[:], lhsT=pt2_s[:], rhs=rsumb[:], start=True, stop=True)
            alpha = per.tile([128, NT], BF, tag="alpha")
            nc.vector.tensor_tensor(out=alpha[:], in0=ee[:], in1=sb_ps[:], op=OP.mult)

            srg_ps = psG.tile([128, 128], F32, tag="pSRG", space="PSUM")
            aptA = per.tile([128, NT, SPT], BF, tag="aptA")
            nc.vector.tensor_tensor(out=aptA[:],
                                    in0=ptf_s[:, None, :].to_broadcast([128, NT, SPT]),
                                    in1=alpha[:, :, None].to_broadcast([128, NT, SPT]),
                                    op=OP.mult)
            for j in range(NT):
                s0 = j * SPT
                nc.tensor.matmul(out=srg_ps[:, s0:s0 + SPT], lhsT=H[:, j, :], rhs=aptA[:, j, :],
                                 start=True, stop=True, skip_group_check=True)
            srgT = per.tile([128, 128], BF, tag="srgT")
            nc.vector.tensor_copy(out=srgT[:], in_=srg_ps[:])
            psr = psE.tile([128, 512], F32, tag="przP", space="PSUM")
            nc.tensor.matmul(out=psr[:, 0:128], lhsT=flT[:], rhs=fsra_s[:],
                             start=True, stop=False, skip_group_check=True)
            nc.tensor.matmul(out=psr[:, 0:128], lhsT=srgT[:], rhs=fsrb_s[:],
                             start=False, stop=True, skip_group_check=True)
            sr = per.tile([128, 128], BF, tag="sr")
            n2s = sc.tile([128, 1], F32, tag="srn2")
            sq1 = sc.tile([128, 128], F32, tag="srsq")
            nc.scalar.activation(out=sq1[:], in_=psr[:, 0:128], func=AF.Square, accum_out=n2s[:])
            nc.vector.tensor_scalar_max(out=n2s[:], in0=n2s[:], scalar1=1e-24)
            recs = rsqrt_dve(n2s, 1, 1.0, F32, "srr")
            nc.vector.tensor_scalar(out=sr[:], in0=psr[:, 0:128], scalar1=recs[:],
                                    scalar2=None, op0=OP.mult)
            srT_ps = psU.tile([128, 256], BF, tag="puP", space="PSUM")
            nc.tensor.transpose(out=srT_ps[:, 0:128], in_=sr[:], identity=id_s[:])
            srT = per.tile([128, 128], BF, tag="srTs")
            nc.vector.tensor_copy(out=srT[:], in_=srT_ps[:, 0:128])

            # ========== logits + log_softmax (own 128 sessions, full vocab) ==========
            # tgt streamed from DRAM once. Chunks alternate storage format in
            # SBUF (state pool space, released here): even chunks keep
            # exp(12L) -> pass 2 = Ln(LOG * 1/Z) on ACT; odd chunks keep raw
            # 12L (DVE cast) -> pass 2 = +(-lnZ) on DVE. Splits pass-2 work
            # across both engines.
            state.release()
            logp = tc.alloc_tile_pool(name="logp", bufs=1)

            def pl_tile(ch):
                if ch % 3 == 0:
                    plt = psE.tile([128, 512], F32, tag="przP", space="PSUM")
                elif ch % 3 == 1:
                    plt = psU.tile([128, 512], F32, tag="puP", space="PSUM")
                else:
                    plt = psA2.tile([128, 512], F32, tag="aggP", space="PSUM")
                return plt

            NCHUNK = (V + 511) // 512
            LOG = logp.tile([128, NCHUNK * 512], BF, tag="LOG")
            separt = per.tile([128, NCHUNK], F32, tag="separt")
            ch = 0
            for t0 in range(0, V, TSC):
                tw = min(TSC, V - t0)
                tg = strm.tile([128, TSC], BF, tag="tgstream")
                nc.sync.dma_start(out=tg[:, :tw], in_=tgtT[:, t0:t0 + tw])
                for q0 in range(0, tw, 512):
                    cw = min(512, tw - q0)
                    pl = pl_tile(ch)
                    MM(out=pl[:, :cw], lhsT=srT[:], rhs=tg[:, q0:q0 + cw],
                       start=True, stop=True)
                    lg = LOG[:, ch * 512:ch * 512 + cw]
                    if ch % 4 == 0:
                        nc.scalar.activation(out=lg, in_=pl[:, :cw], func=AF.Exp,
                                             scale=SCALE, accum_out=separt[:, ch:ch + 1])
                    else:
                        nc.vector.tensor_scalar_mul(out=lg, in0=pl[:, :cw], scalar1=SCALE)
                        escr = ob.tile([128, 512], BF, tag="escr")
                        nc.scalar.activation(out=escr[:, :cw], in_=pl[:, :cw], func=AF.Exp,
                                             scale=SCALE, accum_out=separt[:, ch:ch + 1])
                    ch += 1
            sumexp = per.tile([128, 1], F32, tag="sumexp")
            nc.vector.tensor_reduce(out=sumexp[:], in_=separt[:], axis=AX.X, op=OP.add)
            recz = per.tile([128, 1], F32, tag="recz")
            nc.vector.reciprocal(out=recz[:], in_=sumexp[:])
            nlog = per.tile([128, 1], F32, tag="nlog")
            nc.scalar.activation(out=nlog[:], in_=sumexp[:], func=AF.Ln)
            nc.vector.tensor_scalar_mul(out=nlog[:], in0=nlog[:], scalar1=-1.0)

            OBW = 4096  # output block: 8 chunks, ACT/DVE alternating
            for b0 in range(0, V, OBW):
                bw = min(OBW, V - b0)
                outb = strm.tile([128, OBW], BF, tag="lslB")
                for q0 in range(0, bw, 512):
                    cw = min(512, bw - q0)
                    chq = (b0 + q0) // 512
                    if chq % 4 == 0:
                        nc.scalar.activation(out=outb[:, q0:q0 + cw],
                                             in_=LOG[:, b0 + q0:b0 + q0 + cw],
                                             func=AF.Ln, scale=recz[:])
                    else:
                        nc.vector.tensor_scalar_add(out=outb[:, q0:q0 + cw],
                                                    in0=LOG[:, b0 + q0:b0 + q0 + cw],
                                                    scalar1=nlog[:])
                nc.sync.dma_start(out=out_slice[:, b0:b0 + bw], in_=outb[:, :bw])
            logp.release()

    nc.compile()
    return nc


# ====================== host preprocessing =========================

def prep_inputs(cfg, inputs):
    c = cfg
    V, B, P, NC, PADP = c.V, c.B, c.P, c.NC, c.PADP
    NT, SPT, SC = c.NT, c.SPT, c.SC
    f32 = np.float32

    iid = np.asarray(inputs["iid"]).astype(np.int64)
    esrc = np.asarray(inputs["edge_src"]).astype(np.int64)
    edst = np.asarray(inputs["edge_dst"]).astype(np.int64)
    ew = np.asarray(inputs["edge_w"]).astype(f32)
    et = np.asarray(inputs["edge_t"]).astype(f32)
    emb = np.ascontiguousarray(np.asarray(inputs["embedding"]).astype(f32))
    last_nodes = np.asarray(inputs["last_nodes"]).astype(np.int64)
    assert np.array_equal(last_nodes, np.arange(B) * P + (P - 1)), "unexpected last_nodes"
    es_sess = esrc // P
    assert np.array_equal(es_sess, edst // P), "edges cross sessions"

    dt = float(et.max())
    has_t0 = bool((et <= 0.0).any())

    g = lambda k: np.asarray(inputs[k], f32)
    z0 = 1.0 / (1.0 + np.exp(-(g("bxz") + g("bhz")).astype(np.float64)))
    u0 = np.tanh((g("bxh") + g("bhh")).astype(np.float64))
    omz0 = (1.0 - z0).astype(f32)
    u0 = u0.astype(f32)

    ls = (esrc % P).astype(np.int64)
    ld_ = (edst % P).astype(np.int64)
    no_self = esrc != edst

    Mw = np.zeros((B, PADP, PADP), f32)
    np.add.at(Mw, (es_sess, ls, ld_), ew)
    ws_in = Mw.sum(axis=1)
    ws_out = Mw.sum(axis=2)
    M1T = Mw / np.where(ws_in > 0, ws_in, 1.0)[:, None, :]
    M2T = (Mw / np.where(ws_out > 0, ws_out, 1.0)[:, :, None]).transpose(0, 2, 1)

    def sym_norm(mask):
        Mm = np.zeros((B, PADP, PADP), f32)
        np.add.at(Mm, (es_sess, ls, ld_), mask.astype(f32))
        S = Mm + Mm.transpose(0, 2, 1)
        deg = S.sum(axis=2)
        nrm = np.maximum(deg, 1.0) ** -0.5
        return (nrm[:, :, None] * S * nrm[:, None, :]).astype(f32)

    St_h = sym_norm((et <= np.float32(dt * 0.5)) & no_self)
    St_f = sym_norm((et <= np.float32(dt)) & no_self)
    St_0 = sym_norm((et <= np.float32(0.0)) & no_self) if has_t0 else None

    def blocks_to_tilesT(Bm, core, width=128):
        out = np.zeros((NT, 128, width), f32)
        for s in range(SC):
            j, k = s // SPT, s % SPT
            out[j, k * PADP:(k + 1) * PADP, k * PADP:(k + 1) * PADP] = Bm[core * SC + s]
        return np.ascontiguousarray(out.transpose(1, 0, 2).astype(BF_NP))

    # host-side embedding gather + normalize (input sharding prep)
    feat = emb[iid]
    feat = feat / (np.linalg.norm(feat, axis=1, keepdims=True) + 1e-12)
    featp = np.zeros((B, PADP, 128), f32)
    featp[:, :P, :] = feat.reshape(B, P, 128)
    featp = featp.reshape(NC, SC // SPT, SPT * PADP, 128)  # [NC, NT, 128, 128]

    # normalized target, transposed (full vocab, shared by all cores)
    tgt = emb / (np.linalg.norm(emb, axis=1, keepdims=True) + 1e-12)
    tgtT_full = np.ascontiguousarray(tgt.T.astype(BF_NP))  # [128, V]

    W1, W2 = g("W1"), g("W2")
    gwih, gwhh = g("gru_wih"), g("gru_whh")
    gbih, gbhh = g("gru_bih"), g("gru_bhh")
    P1 = (W1 @ gwih.T[0:256, :]).astype(f32)
    P2 = (W2 @ gwih.T[256:512, :]).astype(f32)
    whhT = np.ascontiguousarray(gwhh.T).copy()
    b_pg = gbih.copy()
    b_pg[0:256] += gbhh[0:256]
    b_h3 = gbhh[256:384].copy()
    # negate z columns so sigmoid(pg[0:256]) = [r | 1-z]
    P1[:, 128:256] *= -1.0
    P2[:, 128:256] *= -1.0
    whhT[:, 128:256] *= -1.0
    b_pg[128:256] *= -1.0

    Wxrz = np.concatenate([g("Wxr"), g("Wxz")], axis=1)
    Whrz = np.concatenate([g("Whr"), g("Whz")], axis=1)
    b_rz = np.concatenate([g("bxr") + g("bhr"), g("bxz") + g("bhz")])
    b_u = g("bxh") + g("bhh")
    # negate z columns -> sigmoid(prz) = [r | 1-z]
    Wxrz[:, 128:256] *= -1.0
    Whrz[:, 128:256] *= -1.0
    b_rz[128:256] *= -1.0

    ptf = np.zeros((128, SPT), f32)
    pt2 = np.zeros((SPT, 128), f32)
    for p in range(128):
        j = p // PADP
        pt2[j, p] = 1.0
        if p % PADP < P:
            ptf[p, j] = 1.0
    # selh[s, T, n] = 1 iff local session s == T*SPT + n//PADP
    selh = np.zeros((128, NT, 128), f32)
    for s in range(128):
        selh[s, s // SPT, (s % SPT) * PADP:(s % SPT + 1) * PADP] = 1.0

    bf = lambda a: np.ascontiguousarray(np.asarray(a, f32).astype(BF_NP))
    shared = dict(
        w_p1=bf(P1), w_p2=bf(P2), w_whhT=bf(whhT),
        w_xrz=bf(Wxrz), w_xh=bf(g("Wxh")), w_hrz=bf(Whrz), w_hh=bf(g("Whh")),
        w_fcu=bf(g("fc_u")), w_fcvw=bf(g("fc_vw")),
        w_fsra=bf(g("fc_sr")[0:128, :]), w_fsrb=bf(g("fc_sr")[128:256, :]),
        b_pgx=bf(np.concatenate([b_pg, b_h3])[None, :]),
        bpg_rep=bf(np.repeat(np.concatenate([b_pg, b_h3])[None, :], 128, axis=0)),
        brz_rep=bf(np.repeat(np.tile(b_rz, 2)[None, :], 128, axis=0)),
        b_rz2=bf(np.tile(b_rz, 2)[None, :]),
        b_u2=bf(np.tile(b_u, 2)[None, :]),
        bvb_row=bf(g("fc_vb")[None, :]),
        ones1=bf(np.ones((1, 128), f32)),
        ptf=bf(ptf), pt2=bf(pt2), sel=bf(selh),
        fce_rep=bf(np.repeat(g("fc_e")[None, :], 128, axis=0)),
        omz0_rep=bf(np.repeat(omz0[None, :], 128, axis=0)),
        u0_rep=bf(np.repeat(u0[None, :], 128, axis=0)),
        identity=bf(np.eye(128, dtype=f32)),
        tgtT=tgtT_full,
    )

    in_maps = []
    for core in range(NC):
        m = dict(shared)
        m["x0"] = np.ascontiguousarray(
            featp[core].transpose(1, 0, 2).astype(BF_NP))  # [128, NT, 128]
        m["x0T"] = np.ascontiguousarray(
            featp[core].transpose(2, 0, 1).astype(BF_NP))  # per-tile transpose
        m["m12tT"] = np.ascontiguousarray(np.concatenate(
            [blocks_to_tilesT(M1T, core), blocks_to_tilesT(M2T, core)], axis=2))
        m["sthT"] = blocks_to_tilesT(St_h, core)
        m["stfT"] = blocks_to_tilesT(St_f, core)
        if has_t0:
            m["st0T"] = blocks_to_tilesT(St_0, core)
        in_maps.append(m)
    return in_maps, dt, has_t0


_NC_CACHE = {}


def kernel(**inputs):
    cfg = FULL
    in_maps, dt, has_t0 = prep_inputs(cfg, inputs)
    key = (round(dt, 9), has_t0)
    if key not in _NC_CACHE:
        _NC_CACHE[key] = build_nc(cfg, dt, has_t0, cfg.NC)
    nc = _NC_CACHE[key]
    res = run_bass_kernel_spmd(nc, in_maps, core_ids=list(range(cfg.NC)),
                               trace=bool(int(os.environ.get("KTRACE", "0"))))
    kernel.last_result = res
    return np.concatenate(
        [np.asarray(res.results[c]["out_slice"]).astype(np.float32)
         for c in range(cfg.NC)], axis=0)


# revision 56
# speedup vs baseline: 1.0062x; 1.0062x over previous
import sys, os
sys.path.insert(0, '/opt/trn_rl_repo')
import numpy as np
import ml_dtypes

import concourse.bass as bass
import concourse.bacc as bacc
import concourse.mybir as mybir
import concourse.tile as tile
from concourse.bass_utils import run_bass_kernel_spmd

F32 = mybir.dt.float32
I32 = mybir.dt.int32
BF = mybir.dt.bfloat16
AF = mybir.ActivationFunctionType
OP = mybir.AluOpType
AX = mybir.AxisListType
SCALE = 12.0
BF_NP = ml_dtypes.bfloat16


class Cfg:
    def __init__(self, V=50000, D=128, B=1024, P=50, NC=8, PADP=64):
        assert D == 128
        self.V, self.D, self.B, self.P, self.NC, self.PADP = V, D, B, P, NC, PADP
        self.SC = B // NC                    # sessions per core
        assert 128 % PADP == 0 and P <= PADP
        self.SPT = 128 // PADP               # sessions per node-tile
        self.NT = self.SC * PADP // 128      # node tiles per core
        assert self.SC == 128                # one session-tile per core


FULL = Cfg()


def build_nc(cfg, dt_val, has_t0, n_cores):
    c = cfg
    NT, SPT, PADP, V = c.NT, c.SPT, c.PADP, c.V
    SCH = 8    # m12t stream chunk (node tiles per dma)
    CH = 16    # stage-tail chunk (tiles)
    TSC = 2048                       # tgt stream columns per dma
    NSTR = (V + TSC - 1) // TSC      # tgt stream steps
    nc = bacc.Bacc("TRN2", target_bir_lowering=False, debug=False, num_devices=n_cores)

    def din(name, shape, dtype=BF):
        return nc.dram_tensor(name, shape, dtype, kind="ExternalInput")

    x0 = din("x0", [128, NT, 128])
    x0T = din("x0T", [128, NT, 128])
    m12tT = din("m12tT", [128, NT, 256])
    sthT = din("sthT", [128, NT, 128])
    stfT = din("stfT", [128, NT, 128])
    st0T = din("st0T", [128, NT, 128]) if has_t0 else None
    tgtT = din("tgtT", [128, V])
    w_p1 = din("w_p1", [128, 384])
    w_p2 = din("w_p2", [128, 384])
    w_whhT = din("w_whhT", [128, 384])
    w_xrz = din("w_xrz", [128, 256])
    w_xh = din("w_xh", [128, 128])
    w_hrz = din("w_hrz", [128, 256])
    w_hh = din("w_hh", [128, 128])
    w_fcu = din("w_fcu", [128, 128])
    w_fcvw = din("w_fcvw", [128, 128])
    w_fsra = din("w_fsra", [128, 128])
    w_fsrb = din("w_fsrb", [128, 128])
    b_pgx = din("b_pgx", [1, 512])   # [b_pg(384) | b_h3(128)]
    bpg_rep = din("bpg_rep", [128, 512])
    brz_rep = din("brz_rep", [128, 512])
    b_rz2 = din("b_rz2", [1, 512])   # [b_rz | b_rz]
    b_u2 = din("b_u2", [1, 256])     # [b_u | b_u]
    bvb_row = din("bvb_row", [1, 128])
    ones1 = din("ones1", [1, 128])
    ptf = din("ptf", [128, SPT])
    pt2 = din("pt2", [SPT, 128])
    sel = din("sel", [128, NT, 128])
    fce_rep = din("fce_rep", [128, 128])
    omz0_rep = din("omz0_rep", [128, 128])
    u0_rep = din("u0_rep", [128, 128])
    identity = din("identity", [128, 128])

    out_slice = nc.dram_tensor("out_slice", [c.SC, V], BF, kind="ExternalOutput")

    dt2 = float(dt_val) * 0.5
    dt6 = float(dt_val) / 6.0
    # NOTE: a single accumulating matmul spanning two separately-started PSUM
    # accumulation groups produced wrong results on HW; bias ones-MMs stay
    # per accumulation group.

    with tile.TileContext(nc) as tc, \
         nc.allow_low_precision("bf16 norm/exp partial sums fine for 2e-2 gate"):
        with tc.tile_pool(name="per", bufs=1) as per, \
             tc.tile_pool(name="str", bufs=2) as strm, \
             tc.tile_pool(name="sc", bufs=3) as sc, \
             tc.tile_pool(name="scd", bufs=5) as scD, \
             tc.tile_pool(name="ob", bufs=4) as ob, \
             tc.tile_pool(name="pse", bufs=2, space="PSUM") as psE, \
             tc.tile_pool(name="psu", bufs=3, space="PSUM") as psU, \
             tc.tile_pool(name="psa", bufs=2, space="PSUM") as psA2, \
             tc.tile_pool(name="psg", bufs=1, space="PSUM") as psG:
            state = tc.alloc_tile_pool(name="state", bufs=1)
            X = state.tile([128, NT, 128], BF, tag="X")
            H = state.tile([128, NT, 128], BF, tag="H")
            KS = state.tile([128, NT, 128], BF, tag="KS")
            DH = state.tile([128, NT, 128], BF, tag="DH")
            SQ = state.tile([128, 16, 128], BF, tag="SQ")  # norm_chunk scratch (CH=16)
            STH = state.tile([128, NT, 128], BF, tag="STH")
            STF = state.tile([128, NT, 128], BF, tag="STF")

            def ld(t, shape, dtype=BF):
                s = per.tile(shape, dtype, tag="c_" + t.name)
                nc.sync.dma_start(out=s[:], in_=t[:])
                return s

            p1_s = ld(w_p1, [128, 384]); p2_s = ld(w_p2, [128, 384])
            whhT_s = ld(w_whhT, [128, 384])
            xrz_s = ld(w_xrz, [128, 256]); xh_s = ld(w_xh, [128, 128])
            hrz_s = ld(w_hrz, [128, 256]); hh_s = ld(w_hh, [128, 128])
            fcu_s = ld(w_fcu, [128, 128]); fcvw_s = ld(w_fcvw, [128, 128])
            fsra_s = ld(w_fsra, [128, 128]); fsrb_s = ld(w_fsrb, [128, 128])
            bpgx_s = ld(b_pgx, [1, 512])
            bpgr_s = ld(bpg_rep, [128, 512])
            brzr_s = ld(brz_rep, [128, 512])
            brz2_s = ld(b_rz2, [1, 512])
            bu2_s = ld(b_u2, [1, 256])
            bvb_s = ld(bvb_row, [1, 128])
            ones_s = ld(ones1, [1, 128])
            ptf_s = ld(ptf, [128, SPT]); pt2_s = ld(pt2, [SPT, 128])
            sel_s = state.tile([128, NT, 128], BF, tag="SEL")
            nc.sync.dma_start(out=sel_s[:], in_=sel[:])
            fce_s = ld(fce_rep, [128, 128])
            id_s = ld(identity, [128, 128])
            omz0_s = u0_s = None
            if not has_t0:
                omz0_s = ld(omz0_rep, [128, 128])
                u0_s = ld(u0_rep, [128, 128])

            # state load first; St matrices issued after the GGNN stream so the
            # m12 chunks aren't queued behind 4MB of eval-time data
            nc.sync.dma_start(out=X[:], in_=x0[:])
            # alias onto KS: KS is first written in the first stage_tail,
            # after the GGNN loop (XT0's last reader) completes
            XT0 = state.tile([128, NT, 128], BF, tag="KS")
            nc.sync.dma_start(out=XT0[:], in_=x0T[:])
            ST0 = None
            if has_t0:
                ST0 = state.tile([128, NT, 128], BF, tag="ST0")

            MM = nc.tensor.matmul

            # ================= GGNN layer =================
            # z-columns of P1/P2/whhT/b_pg are host-negated, so one sigmoid
            # over pg[0:256] yields [r | 1-z]. Software-pipelined one step:
            # agg matmuls a step ahead of the gate chain.
            GNS = NT // 2
            gnp = [None] * GNS; gn12 = [None] * GNS; gpg = [None] * GNS

            def gA(s):  # PE: weighted-mean aggregation
                j = 2 * s
                if s % (SCH // 2) == 0:
                    mt = strm.tile([128, SCH, 256], BF, tag="bigstream", name="mt")
                    jn = min(SCH, NT - j)
                    nc.sync.dma_start(out=mt[:, :jn, :], in_=m12tT[:, j:j + jn, :])
                    gA.mt = mt
                mt = gA.mt
                jj = j % SCH
                a = psA2.tile([128, 512], F32, tag="aggP", space="PSUM", name="nP")
                MM(out=a[:, 0:256], lhsT=X[:, j, :], rhs=mt[:, jj, :],
                   start=True, stop=True, skip_group_check=True)
                MM(out=a[:, 256:512], lhsT=X[:, j + 1, :], rhs=mt[:, jj + 1, :],
                   start=True, stop=True, skip_group_check=True)
                gnp[s] = a

            def gB(s):  # ACT: PSUM -> SBUF
                gn12[s] = sc.tile([128, 512], BF, tag="n12s", name="n12")
                nc.scalar.copy(out=gn12[s][:], in_=gnp[s][:])
                gnp[s] = None

            def gC(s):  # PE: gate matmuls (k=0 bias on PE, k=1 via DVE in gD)
                j = 2 * s
                n12 = gn12[s]
                pgs = []
                for k in range(2):
                    o = 256 * k
                    pool = psE if k == 0 else psU
                    tag = "przP" if k == 0 else "puP"
                    pg = pool.tile([128, 512], F32, tag=tag, space="PSUM", name="pg")
                    MM(out=pg[:, 0:384], lhsT=n12[:, o:o + 128], rhs=p1_s[:],
                       start=True, stop=False, skip_group_check=True)
                    MM(out=pg[:, 0:256], lhsT=XT0[:, j + k, :],
                       rhs=whhT_s[:, 0:256], start=False, stop=False, skip_group_check=True)
                    if k == 0:
                        MM(out=pg[:, 0:384], lhsT=n12[:, o + 128:o + 256], rhs=p2_s[:],
                           start=False, stop=False, skip_group_check=True)
                        MM(out=pg[:, 0:384], lhsT=ones_s[:], rhs=bpgx_s[:, 0:384],
                           start=False, stop=True, skip_group_check=True)
                        MM(out=pg[:, 384:512], lhsT=XT0[:, j + k, :],
                           rhs=whhT_s[:, 256:384], start=True, stop=False, skip_group_check=True)
                        MM(out=pg[:, 384:512], lhsT=ones_s[:], rhs=bpgx_s[:, 384:512],
                           start=False, stop=True, skip_group_check=True)
                    else:
                        MM(out=pg[:, 0:384], lhsT=n12[:, o + 128:o + 256], rhs=p2_s[:],
                           start=False, stop=False, skip_group_check=True)
                        MM(out=pg[:, 0:384], lhsT=ones_s[:], rhs=bpgx_s[:, 0:384],
                           start=False, stop=True, skip_group_check=True)
                        MM(out=pg[:, 384:512], lhsT=XT0[:, j + k, :],
                           rhs=whhT_s[:, 256:384], start=True, stop=False, skip_group_check=True)
                        MM(out=pg[:, 384:512], lhsT=ones_s[:], rhs=bpgx_s[:, 384:512],
                           start=False, stop=True, skip_group_check=True)
                    pgs.append(pg)
                gpg[s] = pgs
                gn12[s] = None

            def gD(s):  # gates + state update
                j = 2 * s
                sigP = sc.tile([128, 2, 256], BF, tag="gsig", name="sigP")
                ntP = sc.tile([128, 2, 128], BF, tag="gnt", name="ntP")
                t2 = sc.tile([128, 2, 128], BF, tag="t1", name="t2")
                for k in range(2):
                    pg = gpg[s][k]
                    nc.scalar.activation(out=sigP[:, k, :], in_=pg[:, 0:256], func=AF.Sigmoid)
                    nc.vector.tensor_tensor(out=t2[:, k, :], in0=sigP[:, k, 0:128],
                                            in1=pg[:, 384:512], op=OP.mult)
                    nc.vector.tensor_tensor(out=t2[:, k, :], in0=t2[:, k, :],
                                            in1=pg[:, 256:384], op=OP.add)
                nc.scalar.activation(out=ntP[:], in_=t2[:], func=AF.Tanh)
                nc.gpsimd.tensor_tensor(out=ntP[:], in0=ntP[:], in1=X[:, j:j + 2, :],
                                        op=OP.subtract)
                nc.gpsimd.tensor_tensor(out=ntP[:], in0=ntP[:], in1=sigP[:, :, 128:256],
                                        op=OP.mult)
                nc.vector.tensor_tensor(out=X[:, j:j + 2, :], in0=X[:, j:j + 2, :],
                                        in1=ntP[:], op=OP.add)
                gpg[s] = None

            for s in range(GNS + 1):
                if s < GNS:
                    gA(s)
                if s >= 1:
                    gC(s - 1)
                if s < GNS:
                    gB(s)
                if s >= 1:
                    gD(s - 1)

            # eval-time data: issued after the GGNN stream DMAs
            nc.sync.dma_start(out=STH[:], in_=sthT[:])
            nc.sync.dma_start(out=STF[:], in_=stfT[:])
            if has_t0:
                nc.sync.dma_start(out=ST0[:], in_=st0T[:])

            def rsqrt_dve(n2, W, cmul, outdt, tag):
                """cs = cmul * n2**-0.5 on DVE only (bit trick + 2 Newton iters;
                ~5e-6 rel err). Avoids ACT Sqrt table thrash vs sigmoid/tanh."""
                ii = sc.tile([128, W], I32, tag=tag + "_i", name="ii")
                nc.vector.tensor_scalar(out=ii[:], in0=n2[:].bitcast(I32), scalar1=1,
                                        scalar2=None, op0=OP.logical_shift_right)
                nc.vector.tensor_scalar(out=ii[:], in0=ii[:], scalar1=0x5f3759df,
                                        scalar2=-1, op0=OP.subtract, op1=OP.mult)
                y0 = ii[:].bitcast(F32)
                t1 = sc.tile([128, W], F32, tag=tag + "_t", name="t1")
                y = sc.tile([128, W], F32, tag=tag + "_y", name="y")
                nc.vector.tensor_tensor(out=t1[:], in0=y0, in1=y0, op=OP.mult)
                nc.vector.tensor_tensor(out=t1[:], in0=t1[:], in1=n2[:], op=OP.mult)
                nc.vector.tensor_scalar(out=t1[:], in0=t1[:], scalar1=-0.5, scalar2=1.5,
                                        op0=OP.mult, op1=OP.add)
                nc.vector.tensor_tensor(out=y[:], in0=t1[:], in1=y0, op=OP.mult)
                nc.vector.tensor_tensor(out=t1[:], in0=y[:], in1=y[:], op=OP.mult)
                nc.vector.tensor_tensor(out=t1[:], in0=t1[:], in1=n2[:], op=OP.mult)
                nc.vector.tensor_scalar(out=t1[:], in0=t1[:],
                                        scalar1=-0.5 * cmul, scalar2=1.5 * cmul,
                                        op0=OP.mult, op1=OP.add)
                cs = sc.tile([128, W], outdt, tag=tag + "_c", name="cs")
                nc.vector.tensor_tensor(out=cs[:], in0=t1[:], in1=y[:], op=OP.mult)
                return cs

            def norm_chunk(arr, c0, eps, cmul):
                """cs[:, c0:c0+CH] = cmul / max(|row|, eps) for one CH-tile chunk.
                All-DVE: squares+reduce then bit-trick rsqrt."""
                n2 = sc.tile([128, CH], F32, tag="nrm_n2", name="n2")
                nc.vector.tensor_tensor(out=SQ[:, :CH, :], in0=arr[:, c0:c0 + CH, :],
                                        in1=arr[:, c0:c0 + CH, :], op=OP.mult)
                nc.vector.tensor_reduce(out=n2[:], in_=SQ[:, :CH, :],
                                        axis=AX.X, op=OP.add)
                nc.vector.tensor_scalar_max(out=n2[:], in0=n2[:], scalar1=max(eps * eps, 1e-37))
                return rsqrt_dve(n2, CH, cmul, BF, "nrm")

            # ================= ODE: RK4 =================
            first_ks = [True]

            def tail_chunks(c_stage, rho, last):
                f = float(rho) / float(c_stage)
                first = first_ks[0]
                first_ks[0] = False

                def mk(c0):
                    def run():
                        c1 = c0 + CH
                        cs = norm_chunk(DH, c0, 1e-12, c_stage)
                        nc.vector.tensor_tensor(
                            out=DH[:, c0:c1, :], in0=DH[:, c0:c1, :],
                            in1=cs[:, :, None].to_broadcast([128, CH, 128]),
                            op=OP.mult)
                        if not last:
                            nc.gpsimd.tensor_tensor(out=H[:, c0:c1, :],
                                                    in0=X[:, c0:c1, :],
                                                    in1=DH[:, c0:c1, :], op=OP.add)
                        if first:
                            nc.vector.tensor_scalar_mul(out=KS[:, c0:c1, :],
                                                        in0=DH[:, c0:c1, :], scalar1=f)
                        else:
                            nc.vector.scalar_tensor_tensor(
                                out=KS[:, c0:c1, :], in0=DH[:, c0:c1, :], scalar=f,
                                in1=KS[:, c0:c1, :], op0=OP.mult, op1=OP.add)
                    return run
                return [mk(c0) for c0 in range(0, NT, CH)]

            def first_stage_chunks():
                """X normalize + t=0 shortcut dh + first tail, per chunk."""
                tl = tail_chunks(dt2, dt6, False)

                def mk(c0):
                    def run():
                        c1 = c0 + CH
                        cs = norm_chunk(X, c0, 1e-12, 1.0)
                        nc.vector.tensor_tensor(
                            out=X[:, c0:c1, :], in0=X[:, c0:c1, :],
                            in1=cs[:, :, None].to_broadcast([128, CH, 128]),
                            op=OP.mult)
                        nc.vector.tensor_tensor(
                            out=DH[:, c0:c1, :],
                            in0=u0_s[:, None, :].to_broadcast([128, CH, 128]),
                            in1=X[:, c0:c1, :], op=OP.subtract)
                        nc.gpsimd.tensor_tensor(
                            out=DH[:, c0:c1, :], in0=DH[:, c0:c1, :],
                            in1=omz0_s[:, None, :].to_broadcast([128, CH, 128]),
                            op=OP.mult)
                        tl[c0 // CH]()
                    return run
                return [mk(c0) for c0 in range(0, NT, CH)]

            def full_eval(st_res, pre):
                # z-columns of xrz/hrz/b_rz host-negated -> sigmoid gives [r | 1-z].
                # Software-pipelined: stage deps are >=1 step old so each
                # engine queue never head-of-line blocks.
                NS = NT // 2
                agg = [None] * NS; prz = [None] * NS; pu = [None] * NS
                sxt = [None] * NS; ghT = [None] * NS; sig = [None] * NS
                rh = [None] * NS; uT = [None] * NS

                def stA(s):  # PE: aggregation matmuls
                    j = 2 * s
                    a = psA2.tile([128, 512], F32, tag="aggP", space="PSUM")
                    MM(out=a[:, 0:128], lhsT=X[:, j, :], rhs=st_res[:, j, :],
                       start=True, stop=True, skip_group_check=True)
                    MM(out=a[:, 128:256], lhsT=X[:, j + 1, :], rhs=st_res[:, j + 1, :],
                       start=True, stop=True, skip_group_check=True)
                    MM(out=a[:, 256:384], lhsT=H[:, j, :], rhs=st_res[:, j, :],
                       start=True, stop=True, skip_group_check=True)
                    MM(out=a[:, 384:512], lhsT=H[:, j + 1, :], rhs=st_res[:, j + 1, :],
                       start=True, stop=True, skip_group_check=True)
                    agg[s] = a

                def stB(s):  # ACT+DVE: PSUM -> SBUF gate inputs
                    sxt[s] = scD.tile([128, 256], BF, tag="sxt", name="sxt")
                    nc.scalar.copy(out=sxt[s][:], in_=agg[s][:, 0:256])
                    ghT[s] = sc.tile([128, 256], BF, tag="ghT", name="ghT")
                    nc.vector.tensor_copy(out=ghT[s][:], in_=agg[s][:, 256:512])

                def stC(s):  # PE: r/z gate matmuls
                    p = psE.tile([128, 512], F32, tag="przP", space="PSUM")
                    for k in range(2):
                        o = 256 * k
                        MM(out=p[:, o:o + 256], lhsT=ghT[s][:, 128 * k:128 * (k + 1)],
                           rhs=hrz_s[:], start=True, stop=False, skip_group_check=True)
                        MM(out=p[:, o:o + 256], lhsT=sxt[s][:, 128 * k:128 * (k + 1)],
                           rhs=xrz_s[:], start=False, stop=False, skip_group_check=True)
                        MM(out=p[:, o:o + 256], lhsT=ones_s[:], rhs=brz2_s[:, 0:256],
                           start=False, stop=True, skip_group_check=True)
                    prz[s] = p
                    ghT[s] = None

                def stD(s):  # ACT sigmoid
                    sig[s] = scD.tile([128, 4, 128], BF, tag="sig", name="sig")
                    nc.scalar.activation(out=sig[s][:], in_=prz[s][:], func=AF.Sigmoid)
                    prz[s] = None

                def stE(s):  # DVE: r * h
                    j = 2 * s
                    rh[s] = sc.tile([128, 2, 128], BF, tag="rh", name="rh")
                    nc.vector.tensor_tensor(out=rh[s][:], in0=sig[s][:, 0::2, :],
                                            in1=H[:, j:j + 2, :], op=OP.mult)

                def stF(s):  # PE: (r*h) aggregation matmuls
                    j = 2 * s
                    p = psU.tile([128, 512], F32, tag="puP", space="PSUM")
                    MM(out=p[:, 0:128], lhsT=rh[s][:, 0, :], rhs=st_res[:, j, :],
                       start=True, stop=True, skip_group_check=True)
                    MM(out=p[:, 128:256], lhsT=rh[s][:, 1, :], rhs=st_res[:, j + 1, :],
                       start=True, stop=True, skip_group_check=True)
                    pu[s] = p
                    rh[s] = None

                def stG(s):  # ACT: PSUM -> SBUF for u-gate lhsT
                    uT[s] = sc.tile([128, 256], BF, tag="uT", name="uT")
                    nc.scalar.copy(out=uT[s][:], in_=pu[s][:, 0:256])

                def stH(s):  # PE: u gate matmuls
                    for k in range(2):
                        o = 256 + 128 * k
                        MM(out=pu[s][:, o:o + 128], lhsT=uT[s][:, 128 * k:128 * (k + 1)],
                           rhs=hh_s[:], start=True, stop=False, skip_group_check=True)
                        MM(out=pu[s][:, o:o + 128], lhsT=sxt[s][:, 128 * k:128 * (k + 1)],
                           rhs=xh_s[:], start=False, stop=False, skip_group_check=True)
                        MM(out=pu[s][:, o:o + 128], lhsT=ones_s[:], rhs=bu2_s[:, 0:128],
                           start=False, stop=True, skip_group_check=True)
                    uT[s] = None
                    sxt[s] = None

                def stI(s):  # ACT tanh + Pool epilogue -> DH
                    j = 2 * s
                    uP = sc.tile([128, 2, 128], BF, tag="ut")
                    nc.scalar.activation(out=uP[:], in_=pu[s][:, 256:512], func=AF.Tanh)
                    nc.gpsimd.tensor_tensor(out=uP[:], in0=uP[:], in1=H[:, j:j + 2, :],
                                            op=OP.subtract)
                    nc.gpsimd.tensor_tensor(out=DH[:, j:j + 2, :], in0=uP[:],
                                            in1=sig[s][:, 1::2, :], op=OP.mult)
                    pu[s] = None
                    sig[s] = None
                    agg[s] = None

                for s in range(NS + 3):
                    if s % (CH // 2) == 0 and s // (CH // 2) < len(pre):
                        pre[s // (CH // 2)]()
                    if s < NS:
                        stA(s)
                    if s >= 1 and s - 1 < NS:
                        stC(s - 1)
                    if s >= 2 and s - 2 < NS:
                        stF(s - 2)
                    if s >= 3 and s - 3 < NS:
                        stH(s - 3)
                    if s < NS:
                        stB(s)
                    if s >= 1 and s - 1 < NS:
                        stD(s - 1)
                        stE(s - 1)
                    if s >= 2 and s - 2 < NS:
                        stG(s - 2)
                    if s >= 3 and s - 3 < NS:
                        stI(s - 3)

            if has_t0:
                # ST0 path needs H initialized to X (f(0, feat) uses h=feat)
                def init_chunks():
                    def mk(c0):
                        def run():
                            c1 = c0 + CH
                            cs = norm_chunk(X, c0, 1e-12, 1.0)
                            nc.vector.tensor_tensor(
                                out=X[:, c0:c1, :], in0=X[:, c0:c1, :],
                                in1=cs[:, :, None].to_broadcast([128, CH, 128]),
                                op=OP.mult)
                            nc.vector.tensor_copy(out=H[:, c0:c1, :], in_=X[:, c0:c1, :])
                        return run
                    return [mk(c0) for c0 in range(0, NT, CH)]
                full_eval(ST0, init_chunks())
                full_eval(STH, tail_chunks(dt2, dt6, False))
            else:
                full_eval(STH, first_stage_chunks())
            full_eval(STH, tail_chunks(dt2, 2.0 * dt6, False))
            full_eval(STF, tail_chunks(float(dt_val), 2.0 * dt6, False))
            # final tail + H = normalize(X + KS), interleaved with readout
            # transposes chunk by chunk
            final_tail = tail_chunks(1.0, dt6, True)
            XT = state.tile([128, NT, 128], BF, tag="X")  # X dead after H formed
            flT = per.tile([128, 128], BF, tag="flTs")
            for c0 in range(0, NT, CH):
                c1 = c0 + CH
                final_tail[c0 // CH]()
                nc.vector.tensor_tensor(out=H[:, c0:c1, :], in0=X[:, c0:c1, :],
                                        in1=KS[:, c0:c1, :], op=OP.add)
                cs = norm_chunk(H, c0, 1e-30, 1.0)
                nc.vector.tensor_tensor(out=H[:, c0:c1, :], in0=H[:, c0:c1, :],
                                        in1=cs[:, :, None].to_broadcast([128, CH, 128]),
                                        op=OP.mult)
                for j in range(c0, c1, 2):
                    xtP = psU.tile([128, 256], BF, tag="puP", space="PSUM")
                    nc.tensor.transpose(out=xtP[:, 0:128], in_=H[:, j, :], identity=id_s[:])
                    nc.tensor.transpose(out=xtP[:, 128:256], in_=H[:, j + 1, :], identity=id_s[:])
                    if (j // 2) % 2 == 0:
                        nc.scalar.copy(out=XT[:, j:j + 2, :], in_=xtP[:])
                    else:
                        nc.vector.tensor_copy(out=XT[:, j:j + 2, :], in_=xtP[:])
                    nc.vector.tensor_copy(out=flT[:, j * SPT:(j + 2) * SPT],
                                          in_=XT[:, j:j + 2, c.P - 1::PADP])
            # fvS[sess, d] = H_last @ fc_vw + fc_vb   (sessions on partitions)
            pfv = psA2.tile([128, 512], F32, tag="aggP", space="PSUM")
            nc.tensor.matmul(out=pfv[:, 0:128], lhsT=flT[:], rhs=fcvw_s[:],
                             start=True, stop=False, skip_group_check=True)
            nc.tensor.matmul(out=pfv[:, 0:128], lhsT=ones_s[:], rhs=bvb_s[:],
                             start=False, stop=True, skip_group_check=True)
            fvS = per.tile([128, 128], BF, tag="fvS")
            nc.scalar.copy(out=fvS[:], in_=pfv[:, 0:128])

            ee = per.tile([128, NT], BF, tag="ee")
            ecolF = per.tile([128, NT], F32, tag="ecolF")
            for j in range(0, NT, 2):
                peP = psA2.tile([128, 512], F32, tag="aggP", space="PSUM")
                for k in range(2):
                    o = 128 * k
                    s0 = (j + k) * SPT
                    MM(out=peP[:, o:o + 128], lhsT=XT[:, j + k, :], rhs=fcu_s[:],
                       start=True, stop=False, skip_group_check=True)
                    # selection matmul: one-hot [sess, node] map adds fv[sess(n), :]
                    MM(out=peP[:, o:o + 128], lhsT=sel_s[:, j + k, :], rhs=fvS[:],
                       start=False, stop=True, skip_group_check=True)
                sg = sc.tile([128, 2, 128], BF, tag="sg")
                nc.scalar.activation(out=sg[:], in_=peP[:, 0:256], func=AF.Sigmoid)
                nc.vector.tensor_tensor(out=sg[:], in0=sg[:],
                                        in1=fce_s[:, None, :].to_broadcast([128, 2, 128]),
                                        op=OP.mult)
                nc.vector.tensor_reduce(out=ecolF[:, j:j + 2], in_=sg[:], axis=AX.X, op=OP.add)
            nc.scalar.activation(out=ee[:], in_=ecolF[:], func=AF.Exp)
            ssum_ps = psE.tile([SPT, NT], F32, tag="przP", space="PSUM")
            nc.tensor.matmul(out=ssum_ps[:], lhsT=ptf_s[:], rhs=ee[:], start=True, stop=True)
            rsum = per.tile([SPT, NT], F32, tag="rsum")
            nc.vector.reciprocal(out=rsum[:], in_=ssum_ps[:])
            rsumb = per.tile([SPT, NT], BF, tag="rsumb")
            nc.vector.tensor_copy(out=rsumb[:], in_=rsum[:])
            sb_ps = psE.tile([128, NT], F32, tag="przP", space="PSUM")
            nc.tensor.matmul(out=sb_ps[:], lhsT=pt2_s[:], rhs=rsumb[:], start=True, stop=True)
            alpha = per.tile([128, NT], BF, tag="alpha")
            nc.vector.tensor_tensor(out=alpha[:], in0=ee[:], in1=sb_ps[:], op=OP.mult)

            srg_ps = psG.tile([128, 128], F32, tag="pSRG", space="PSUM")
            aptA = per.tile([128, NT, SPT], BF, tag="aptA")
            nc.vector.tensor_tensor(out=aptA[:],
                                    in0=ptf_s[:, None, :].to_broadcast([128, NT, SPT]),
                                    in1=alpha[:, :, None].to_broadcast([128, NT, SPT]),
                                    op=OP.mult)
            for j in range(NT):
                s0 = j * SPT
                nc.tensor.matmul(out=srg_ps[:, s0:s0 + SPT], lhsT=H[:, j, :], rhs=aptA[:, j, :],
                                 start=True, stop=True, skip_group_check=True)
            srgT = per.tile([128, 128], BF, tag="srgT")
            nc.vector.tensor_copy(out=srgT[:], in_=srg_ps[:])
            psr = psE.tile([128, 512], F32, tag="przP", space="PSUM")
            nc.tensor.matmul(out=psr[:, 0:128], lhsT=flT[:], rhs=fsra_s[:],
                             start=True, stop=False, skip_group_check=True)
            nc.tensor.matmul(out=psr[:, 0:128], lhsT=srgT[:], rhs=fsrb_s[:],
                             start=False, stop=True, skip_group_check=True)
            sr = per.tile([128, 128], BF, tag="sr")
            n2s = sc.tile([128, 1], F32, tag="srn2")
            sq1 = sc.tile([128, 128], F32, tag="srsq")
            nc.scalar.activation(out=sq1[:], in_=psr[:, 0:128], func=AF.Square, accum_out=n2s[:])
            nc.vector.tensor_scalar_max(out=n2s[:], in0=n2s[:], scalar1=1e-24)
            recs = rsqrt_dve(n2s, 1, 1.0, F32, "srr")
            nc.vector.tensor_scalar(out=sr[:], in0=psr[:, 0:128], scalar1=recs[:],
                                    scalar2=None, op0=OP.mult)
            srT_ps = psU.tile([128, 256], BF, tag="puP", space="PSUM")
            nc.tensor.transpose(out=srT_ps[:, 0:128], in_=sr[:], identity=id_s[:])
            srT = per.tile([128, 128], BF, tag="srTs")
            nc.vector.tensor_copy(out=srT[:], in_=srT_ps[:, 0:128])

            # ========== logits + log_softmax (own 128 sessions, full vocab) ==========
            # tgt streamed from DRAM once. Chunks alternate storage format in
            # SBUF (state pool space, released here): even chunks keep
            # exp(12L) -> pass 2 = Ln(LOG * 1/Z) on ACT; odd chunks keep raw
            # 12L (DVE cast) -> pass 2 = +(-lnZ) on DVE. Splits pass-2 work
            # across both engines.
            state.release()
            logp = tc.alloc_tile_pool(name="logp", bufs=1)

            def pl_tile(ch):
                if ch % 3 == 0:
                    plt = psE.tile([128, 512], F32, tag="przP", space="PSUM")
                elif ch % 3 == 1:
                    plt = psU.tile([128, 512], F32, tag="puP", space="PSUM")
                else:
                    plt = psA2.tile([128, 512], F32, tag="aggP", space="PSUM")
                return plt

            NCHUNK = (V + 511) // 512
            LOG = logp.tile([128, NCHUNK * 512], BF, tag="LOG")
            separt = per.tile([128, NCHUNK], F32, tag="separt")
            ch = 0
            for t0 in range(0, V, TSC):
                tw = min(TSC, V - t0)
                tg = strm.tile([128, TSC], BF, tag="tgstream")
                nc.sync.dma_start(out=tg[:, :tw], in_=tgtT[:, t0:t0 + tw])
                for q0 in range(0, tw, 512):
                    cw = min(512, tw - q0)
                    pl = pl_tile(ch)
                    MM(out=pl[:, :cw], lhsT=srT[:], rhs=tg[:, q0:q0 + cw],
                       start=True, stop=True)
                    lg = LOG[:, ch * 512:ch * 512 + cw]
                    if ch % 4 == 0:
                        nc.scalar.activation(out=lg, in_=pl[:, :cw], func=AF.Exp,
                                             scale=SCALE, accum_out=separt[:, ch:ch + 1])
                    else:
                        nc.vector.tensor_scalar_mul(out=lg, in0=pl[:, :cw], scalar1=SCALE)
                        escr = ob.tile([128, 512], BF, tag="escr")
                        nc.scalar.activation(out=escr[:, :cw], in_=pl[:, :cw], func=AF.Exp,
                                             scale=SCALE, accum_out=separt[:, ch:ch + 1])
                    ch += 1
            sumexp = per.tile([128, 1], F32, tag="sumexp")
            nc.vector.tensor_reduce(out=sumexp[:], in_=separt[:], axis=AX.X, op=OP.add)
            recz = per.tile([128, 1], F32, tag="recz")
            nc.vector.reciprocal(out=recz[:], in_=sumexp[:])
            nlog = per.tile([128, 1], F32, tag="nlog")
            nc.scalar.activation(out=nlog[:], in_=sumexp[:], func=AF.Ln)
            nc.vector.tensor_scalar_mul(out=nlog[:], in0=nlog[:], scalar1=-1.0)

            OBW = 4096  # output block: 8 chunks, ACT/DVE alternating
            for b0 in range(0, V, OBW):
                bw = min(OBW, V - b0)
                outb = strm.tile([128, OBW], BF, tag="lslB")
                for q0 in range(0, bw, 512):
                    cw = min(512, bw - q0)
                    chq = (b0 + q0) // 512
                    if chq % 4 == 0:
                        nc.scalar.activation(out=outb[:, q0:q0 + cw],
                                             in_=LOG[:, b0 + q0:b0 + q0 + cw],
                                             func=AF.Ln, scale=recz[:])
                    else:
                        nc.vector.tensor_scalar_add(out=outb[:, q0:q0 + cw],
                                                    in0=LOG[:, b0 + q0:b0 + q0 + cw],
                                                    scalar1=nlog[:])
                nc.sync.dma_start(out=out_slice[:, b0:b0 + bw], in_=outb[:, :bw])
            logp.release()

    nc.compile()
    return nc


# ====================== host preprocessing =========================

def prep_inputs(cfg, inputs):
    c = cfg
    V, B, P, NC, PADP = c.V, c.B, c.P, c.NC, c.PADP
    NT, SPT, SC = c.NT, c.SPT, c.SC
    f32 = np.float32

    iid = np.asarray(inputs["iid"]).astype(np.int64)
    esrc = np.asarray(inputs["edge_src"]).astype(np.int64)
    edst = np.asarray(inputs["edge_dst"]).astype(np.int64)
    ew = np.asarray(inputs["edge_w"]).astype(f32)
    et = np.asarray(inputs["edge_t"]).astype(f32)
    emb = np.ascontiguousarray(np.asarray(inputs["embedding"]).astype(f32))
    last_nodes = np.asarray(inputs["last_nodes"]).astype(np.int64)
    assert np.array_equal(last_nodes, np.arange(B) * P + (P - 1)), "unexpected last_nodes"
    es_sess = esrc // P
    assert np.array_equal(es_sess, edst // P), "edges cross sessions"

    dt = float(et.max())
    has_t0 = bool((et <= 0.0).any())

    g = lambda k: np.asarray(inputs[k], f32)
    z0 = 1.0 / (1.0 + np.exp(-(g("bxz") + g("bhz")).astype(np.float64)))
    u0 = np.tanh((g("bxh") + g("bhh")).astype(np.float64))
    omz0 = (1.0 - z0).astype(f32)
    u0 = u0.astype(f32)

    ls = (esrc % P).astype(np.int64)
    ld_ = (edst % P).astype(np.int64)
    no_self = esrc != edst

    Mw = np.zeros((B, PADP, PADP), f32)
    np.add.at(Mw, (es_sess, ls, ld_), ew)
    ws_in = Mw.sum(axis=1)
    ws_out = Mw.sum(axis=2)
    M1T = Mw / np.where(ws_in > 0, ws_in, 1.0)[:, None, :]
    M2T = (Mw / np.where(ws_out > 0, ws_out, 1.0)[:, :, None]).transpose(0, 2, 1)

    def sym_norm(mask):
        Mm = np.zeros((B, PADP, PADP), f32)
        np.add.at(Mm, (es_sess, ls, ld_), mask.astype(f32))
        S = Mm + Mm.transpose(0, 2, 1)
        deg = S.sum(axis=2)
        nrm = np.maximum(deg, 1.0) ** -0.5
        return (nrm[:, :, None] * S * nrm[:, None, :]).astype(f32)

    St_h = sym_norm((et <= np.float32(dt * 0.5)) & no_self)
    St_f = sym_norm((et <= np.float32(dt)) & no_self)
    St_0 = sym_norm((et <= np.float32(0.0)) & no_self) if has_t0 else None

    def blocks_to_tilesT(Bm, core, width=128):
        out = np.zeros((NT, 128, width), f32)
        for s in range(SC):
            j, k = s // SPT, s % SPT
            out[j, k * PADP:(k + 1) * PADP, k * PADP:(k + 1) * PADP] = Bm[core * SC + s]
        return np.ascontiguousarray(out.transpose(1, 0, 2).astype(BF_NP))

    # host-side embedding gather + normalize (input sharding prep)
    feat = emb[iid]
    feat = feat / (np.linalg.norm(feat, axis=1, keepdims=True) + 1e-12)
    featp = np.zeros((B, PADP, 128), f32)
    featp[:, :P, :] = feat.reshape(B, P, 128)
    featp = featp.reshape(NC, SC // SPT, SPT * PADP, 128)  # [NC, NT, 128, 128]

    # normalized target, transposed (full vocab, shared by all cores)
    tgt = emb / (np.linalg.norm(emb, axis=1, keepdims=True) + 1e-12)
    tgtT_full = np.ascontiguousarray(tgt.T.astype(BF_NP))  # [128, V]

    W1, W2 = g("W1"), g("W2")
    gwih, gwhh = g("gru_wih"), g("gru_whh")
    gbih, gbhh = g("gru_bih"), g("gru_bhh")
    P1 = (W1 @ gwih.T[0:256, :]).astype(f32)
    P2 = (W2 @ gwih.T[256:512, :]).astype(f32)
    whhT = np.ascontiguousarray(gwhh.T).copy()
    b_pg = gbih.copy()
    b_pg[0:256] += gbhh[0:256]
    b_h3 = gbhh[256:384].copy()
    # negate z columns so sigmoid(pg[0:256]) = [r | 1-z]
    P1[:, 128:256] *= -1.0
    P2[:, 128:256] *= -1.0
    whhT[:, 128:256] *= -1.0
    b_pg[128:256] *= -1.0

    Wxrz = np.concatenate([g("Wxr"), g("Wxz")], axis=1)
    Whrz = np.concatenate([g("Whr"), g("Whz")], axis=1)
    b_rz = np.concatenate([g("bxr") + g("bhr"), g("bxz") + g("bhz")])
    b_u = g("bxh") + g("bhh")
    # negate z columns -> sigmoid(prz) = [r | 1-z]
    Wxrz[:, 128:256] *= -1.0
    Whrz[:, 128:256] *= -1.0
    b_rz[128:256] *= -1.0

    ptf = np.zeros((128, SPT), f32)
    pt2 = np.zeros((SPT, 128), f32)
    for p in range(128):
        j = p // PADP
        pt2[j, p] = 1.0
        if p % PADP < P:
            ptf[p, j] = 1.0
    # selh[s, T, n] = 1 iff local session s == T*SPT + n//PADP
    selh = np.zeros((128, NT, 128), f32)
    for s in range(128):
        selh[s, s // SPT, (s % SPT) * PADP:(s % SPT + 1) * PADP] = 1.0

    bf = lambda a: np.ascontiguousarray(np.asarray(a, f32).astype(BF_NP))
    shared = dict(
        w_p1=bf(P1), w_p2=bf(P2), w_whhT=bf(whhT),
        w_xrz=bf(Wxrz), w_xh=bf(g("Wxh")), w_hrz=bf(Whrz), w_hh=bf(g("Whh")),
        w_fcu=bf(g("fc_u")), w_fcvw=bf(g("fc_vw")),
        w_fsra=bf(g("fc_sr")[0:128, :]), w_fsrb=bf(g("fc_sr")[128:256, :]),
        b_pgx=bf(np.concatenate([b_pg, b_h3])[None, :]),
        bpg_rep=bf(np.repeat(np.concatenate([b_pg, b_h3])[None, :], 128, axis=0)),
        brz_rep=bf(np.repeat(np.tile(b_rz, 2)[None, :], 128, axis=0)),
        b_rz2=bf(np.tile(b_rz, 2)[None, :]),
        b_u2=bf(np.tile(b_u, 2)[None, :]),
        bvb_row=bf(g("fc_vb")[None, :]),
        ones1=bf(np.ones((1, 128), f32)),
        ptf=bf(ptf), pt2=bf(pt2), sel=bf(selh),
        fce_rep=bf(np.repeat(g("fc_e")[None, :], 128, axis=0)),
        omz0_rep=bf(np.repeat(omz0[None, :], 128, axis=0)),
        u0_rep=bf(np.repeat(u0[None, :], 128, axis=0)),
        identity=bf(np.eye(128, dtype=f32)),
        tgtT=tgtT_full,
    )

    in_maps = []
    for core in range(NC):
        m = dict(shared)
        m["x0"] = np.ascontiguousarray(
            featp[core].transpose(1, 0, 2).astype(BF_NP))  # [128, NT, 128]
        m["x0T"] = np.ascontiguousarray(
            featp[core].transpose(2, 0, 1).astype(BF_NP))  # per-tile transpose
        m["m12tT"] = np.ascontiguousarray(np.concatenate(
            [blocks_to_tilesT(M1T, core), blocks_to_tilesT(M2T, core)], axis=2))
        m["sthT"] = blocks_to_tilesT(St_h, core)
        m["stfT"] = blocks_to_tilesT(St_f, core)
        if has_t0:
            m["st0T"] = blocks_to_tilesT(St_0, core)
        in_maps.append(m)
    return in_maps, dt, has_t0


_NC_CACHE = {}


def kernel(**inputs):
    cfg = FULL
    in_maps, dt, has_t0 = prep_inputs(cfg, inputs)
    key = (round(dt, 9), has_t0)
    if key not in _NC_CACHE:
        _NC_CACHE[key] = build_nc(cfg, dt, has_t0, cfg.NC)
    nc = _NC_CACHE[key]
    res = run_bass_kernel_spmd(nc, in_maps, core_ids=list(range(cfg.NC)),
                               trace=bool(int(os.environ.get("KTRACE", "0"))))
    kernel.last_result = res
    return np.concatenate(
        [np.asarray(res.results[c]["out_slice"]).astype(np.float32)
         for c in range(cfg.NC)], axis=0)


# revision 57
# speedup vs baseline: 1.0297x; 1.0234x over previous
import sys, os
sys.path.insert(0, '/opt/trn_rl_repo')
import numpy as np
import ml_dtypes

import concourse.bass as bass
import concourse.bacc as bacc
import concourse.mybir as mybir
import concourse.tile as tile
from concourse.bass_utils import run_bass_kernel_spmd

F32 = mybir.dt.float32
I32 = mybir.dt.int32
BF = mybir.dt.bfloat16
AF = mybir.ActivationFunctionType
OP = mybir.AluOpType
AX = mybir.AxisListType
SCALE = 12.0
BF_NP = ml_dtypes.bfloat16


class Cfg:
    def __init__(self, V=50000, D=128, B=1024, P=50, NC=8, PADP=64):
        assert D == 128
        self.V, self.D, self.B, self.P, self.NC, self.PADP = V, D, B, P, NC, PADP
        self.SC = B // NC                    # sessions per core
        assert 128 % PADP == 0 and P <= PADP
        self.SPT = 128 // PADP               # sessions per node-tile
        self.NT = self.SC * PADP // 128      # node tiles per core
        assert self.SC == 128                # one session-tile per core


FULL = Cfg()


def build_nc(cfg, dt_val, has_t0, n_cores):
    c = cfg
    NT, SPT, PADP, V = c.NT, c.SPT, c.PADP, c.V
    SCH = 8    # m12t stream chunk (node tiles per dma)
    CH = 16    # stage-tail chunk (tiles)
    TSC = 2048                       # tgt stream columns per dma
    NSTR = (V + TSC - 1) // TSC      # tgt stream steps
    nc = bacc.Bacc("TRN2", target_bir_lowering=False, debug=False, num_devices=n_cores)

    def din(name, shape, dtype=BF):
        return nc.dram_tensor(name, shape, dtype, kind="ExternalInput")

    x0 = din("x0", [128, NT, 128])
    x0T = din("x0T", [128, NT, 128])
    m12tT = din("m12tT", [128, NT, 256])
    sthT = din("sthT", [128, NT, 128])
    stfT = din("stfT", [128, NT, 128])
    st0T = din("st0T", [128, NT, 128]) if has_t0 else None
    tgtT = din("tgtT", [128, V])
    w_p1 = din("w_p1", [128, 384])
    w_p2 = din("w_p2", [128, 384])
    w_whhT = din("w_whhT", [128, 384])
    w_xrz = din("w_xrz", [128, 256])
    w_xh = din("w_xh", [128, 128])
    w_hrz = din("w_hrz", [128, 256])
    w_hh = din("w_hh", [128, 128])
    w_fcu = din("w_fcu", [128, 128])
    w_fcvw = din("w_fcvw", [128, 128])
    w_fsra = din("w_fsra", [128, 128])
    w_fsrb = din("w_fsrb", [128, 128])
    b_pgx = din("b_pgx", [1, 512])   # [b_pg(384) | b_h3(128)]
    bpg_rep = din("bpg_rep", [128, 512])
    brz_rep = din("brz_rep", [128, 512])
    b_rz2 = din("b_rz2", [1, 512])   # [b_rz | b_rz]
    b_u2 = din("b_u2", [1, 256])     # [b_u | b_u]
    bvb_row = din("bvb_row", [1, 128])
    ones1 = din("ones1", [1, 128])
    ptf = din("ptf", [128, SPT])
    pt2 = din("pt2", [SPT, 128])
    sel = din("sel", [128, NT, 128])
    fce_rep = din("fce_rep", [128, 128])
    omz0_rep = din("omz0_rep", [128, 128])
    u0_rep = din("u0_rep", [128, 128])
    identity = din("identity", [128, 128])

    out_slice = nc.dram_tensor("out_slice", [c.SC, V], BF, kind="ExternalOutput")

    dt2 = float(dt_val) * 0.5
    dt6 = float(dt_val) / 6.0
    # NOTE: a single accumulating matmul spanning two separately-started PSUM
    # accumulation groups produced wrong results on HW; bias ones-MMs stay
    # per accumulation group.

    with tile.TileContext(nc) as tc, \
         nc.allow_low_precision("bf16 norm/exp partial sums fine for 2e-2 gate"):
        with tc.tile_pool(name="per", bufs=1) as per, \
             tc.tile_pool(name="str", bufs=2) as strm, \
             tc.tile_pool(name="sc", bufs=3) as sc, \
             tc.tile_pool(name="scd", bufs=5) as scD, \
             tc.tile_pool(name="ob", bufs=4) as ob, \
             tc.tile_pool(name="pse", bufs=2, space="PSUM") as psE, \
             tc.tile_pool(name="psu", bufs=3, space="PSUM") as psU, \
             tc.tile_pool(name="psa", bufs=2, space="PSUM") as psA2, \
             tc.tile_pool(name="psg", bufs=1, space="PSUM") as psG:
            state = tc.alloc_tile_pool(name="state", bufs=1)
            X = state.tile([128, NT, 128], BF, tag="X")
            H = state.tile([128, NT, 128], BF, tag="H")
            KS = state.tile([128, NT, 128], BF, tag="KS")
            DH = state.tile([128, NT, 128], BF, tag="DH")
            SQ = state.tile([128, 16, 128], BF, tag="SQ")  # norm_chunk scratch (CH=16)
            STH = state.tile([128, NT, 128], BF, tag="STH")
            STF = state.tile([128, NT, 128], BF, tag="STF")

            def ld(t, shape, dtype=BF):
                s = per.tile(shape, dtype, tag="c_" + t.name)
                nc.sync.dma_start(out=s[:], in_=t[:])
                return s

            p1_s = ld(w_p1, [128, 384]); p2_s = ld(w_p2, [128, 384])
            whhT_s = ld(w_whhT, [128, 384])
            xrz_s = ld(w_xrz, [128, 256]); xh_s = ld(w_xh, [128, 128])
            hrz_s = ld(w_hrz, [128, 256]); hh_s = ld(w_hh, [128, 128])
            fcu_s = ld(w_fcu, [128, 128]); fcvw_s = ld(w_fcvw, [128, 128])
            fsra_s = ld(w_fsra, [128, 128]); fsrb_s = ld(w_fsrb, [128, 128])
            bpgx_s = ld(b_pgx, [1, 512])
            bpgr_s = ld(bpg_rep, [128, 512])
            brzr_s = ld(brz_rep, [128, 512])
            brz2_s = ld(b_rz2, [1, 512])
            bu2_s = ld(b_u2, [1, 256])
            bvb_s = ld(bvb_row, [1, 128])
            ones_s = ld(ones1, [1, 128])
            ptf_s = ld(ptf, [128, SPT]); pt2_s = ld(pt2, [SPT, 128])
            sel_s = state.tile([128, NT, 128], BF, tag="SEL")
            nc.sync.dma_start(out=sel_s[:], in_=sel[:])
            fce_s = ld(fce_rep, [128, 128])
            id_s = ld(identity, [128, 128])
            omz0_s = u0_s = None
            if not has_t0:
                omz0_s = ld(omz0_rep, [128, 128])
                u0_s = ld(u0_rep, [128, 128])

            # state load first; St matrices issued after the GGNN stream so the
            # m12 chunks aren't queued behind 4MB of eval-time data
            nc.sync.dma_start(out=X[:], in_=x0[:])
            # alias onto KS: KS is first written in the first stage_tail,
            # after the GGNN loop (XT0's last reader) completes
            XT0 = state.tile([128, NT, 128], BF, tag="KS")
            nc.sync.dma_start(out=XT0[:], in_=x0T[:])
            ST0 = None
            if has_t0:
                ST0 = state.tile([128, NT, 128], BF, tag="ST0")

            MM = nc.tensor.matmul

            # ================= GGNN layer =================
            # z-columns of P1/P2/whhT/b_pg are host-negated, so one sigmoid
            # over pg[0:256] yields [r | 1-z]. Software-pipelined one step:
            # agg matmuls a step ahead of the gate chain.
            GNS = NT // 2
            gnp = [None] * GNS; gn12 = [None] * GNS; gpg = [None] * GNS

            def gA(s):  # PE: weighted-mean aggregation
                j = 2 * s
                if s % (SCH // 2) == 0:
                    mt = strm.tile([128, SCH, 256], BF, tag="bigstream", name="mt")
                    jn = min(SCH, NT - j)
                    nc.sync.dma_start(out=mt[:, :jn, :], in_=m12tT[:, j:j + jn, :])
                    gA.mt = mt
                mt = gA.mt
                jj = j % SCH
                a = psA2.tile([128, 512], F32, tag="aggP", space="PSUM", name="nP")
                MM(out=a[:, 0:256], lhsT=X[:, j, :], rhs=mt[:, jj, :],
                   start=True, stop=True, skip_group_check=True)
                MM(out=a[:, 256:512], lhsT=X[:, j + 1, :], rhs=mt[:, jj + 1, :],
                   start=True, stop=True, skip_group_check=True)
                gnp[s] = a

            def gB(s):  # ACT: PSUM -> SBUF
                gn12[s] = sc.tile([128, 512], BF, tag="n12s", name="n12")
                nc.scalar.copy(out=gn12[s][:], in_=gnp[s][:])
                gnp[s] = None

            def gC(s):  # PE: gate matmuls (k=0 bias on PE, k=1 via DVE in gD)
                j = 2 * s
                n12 = gn12[s]
                pgs = []
                for k in range(2):
                    o = 256 * k
                    pool = psE if k == 0 else psU
                    tag = "przP" if k == 0 else "puP"
                    pg = pool.tile([128, 512], F32, tag=tag, space="PSUM", name="pg")
                    MM(out=pg[:, 0:384], lhsT=n12[:, o:o + 128], rhs=p1_s[:],
                       start=True, stop=False, skip_group_check=True)
                    MM(out=pg[:, 0:256], lhsT=XT0[:, j + k, :],
                       rhs=whhT_s[:, 0:256], start=False, stop=False, skip_group_check=True)
                    if k == 0:
                        MM(out=pg[:, 0:384], lhsT=n12[:, o + 128:o + 256], rhs=p2_s[:],
                           start=False, stop=False, skip_group_check=True)
                        MM(out=pg[:, 0:384], lhsT=ones_s[:], rhs=bpgx_s[:, 0:384],
                           start=False, stop=True, skip_group_check=True)
                        MM(out=pg[:, 384:512], lhsT=XT0[:, j + k, :],
                           rhs=whhT_s[:, 256:384], start=True, stop=False, skip_group_check=True)
                        MM(out=pg[:, 384:512], lhsT=ones_s[:], rhs=bpgx_s[:, 384:512],
                           start=False, stop=True, skip_group_check=True)
                    else:
                        MM(out=pg[:, 0:384], lhsT=n12[:, o + 128:o + 256], rhs=p2_s[:],
                           start=False, stop=False, skip_group_check=True)
                        MM(out=pg[:, 0:384], lhsT=ones_s[:], rhs=bpgx_s[:, 0:384],
                           start=False, stop=True, skip_group_check=True)
                        MM(out=pg[:, 384:512], lhsT=XT0[:, j + k, :],
                           rhs=whhT_s[:, 256:384], start=True, stop=False, skip_group_check=True)
                        MM(out=pg[:, 384:512], lhsT=ones_s[:], rhs=bpgx_s[:, 384:512],
                           start=False, stop=True, skip_group_check=True)
                    pgs.append(pg)
                gpg[s] = pgs
                gn12[s] = None

            def gD(s):  # gates + state update
                j = 2 * s
                sigP = sc.tile([128, 2, 256], BF, tag="gsig", name="sigP")
                ntP = sc.tile([128, 2, 128], BF, tag="gnt", name="ntP")
                t2 = sc.tile([128, 2, 128], BF, tag="t1", name="t2")
                for k in range(2):
                    pg = gpg[s][k]
                    nc.scalar.activation(out=sigP[:, k, :], in_=pg[:, 0:256], func=AF.Sigmoid)
                    nc.vector.tensor_tensor(out=t2[:, k, :], in0=sigP[:, k, 0:128],
                                            in1=pg[:, 384:512], op=OP.mult)
                    nc.vector.tensor_tensor(out=t2[:, k, :], in0=t2[:, k, :],
                                            in1=pg[:, 256:384], op=OP.add)
                nc.scalar.activation(out=ntP[:], in_=t2[:], func=AF.Tanh)
                nc.gpsimd.tensor_tensor(out=ntP[:], in0=ntP[:], in1=X[:, j:j + 2, :],
                                        op=OP.subtract)
                nc.gpsimd.tensor_tensor(out=ntP[:], in0=ntP[:], in1=sigP[:, :, 128:256],
                                        op=OP.mult)
                nc.vector.tensor_tensor(out=X[:, j:j + 2, :], in0=X[:, j:j + 2, :],
                                        in1=ntP[:], op=OP.add)
                gpg[s] = None

            for s in range(GNS + 1):
                if s < GNS:
                    gA(s)
                if s >= 1:
                    gC(s - 1)
                if s < GNS:
                    gB(s)
                if s >= 1:
                    gD(s - 1)

            # eval-time data: issued after the GGNN stream DMAs
            nc.sync.dma_start(out=STH[:], in_=sthT[:])
            nc.sync.dma_start(out=STF[:], in_=stfT[:])
            if has_t0:
                nc.sync.dma_start(out=ST0[:], in_=st0T[:])

            def rsqrt_dve(n2, W, cmul, outdt, tag):
                """cs = cmul * n2**-0.5 on DVE only (bit trick + 2 Newton iters;
                ~5e-6 rel err). Avoids ACT Sqrt table thrash vs sigmoid/tanh."""
                ii = sc.tile([128, W], I32, tag=tag + "_i", name="ii")
                nc.vector.tensor_scalar(out=ii[:], in0=n2[:].bitcast(I32), scalar1=1,
                                        scalar2=None, op0=OP.logical_shift_right)
                nc.vector.tensor_scalar(out=ii[:], in0=ii[:], scalar1=0x5f3759df,
                                        scalar2=-1, op0=OP.subtract, op1=OP.mult)
                y0 = ii[:].bitcast(F32)
                t1 = sc.tile([128, W], F32, tag=tag + "_t", name="t1")
                y = sc.tile([128, W], F32, tag=tag + "_y", name="y")
                nc.vector.tensor_tensor(out=t1[:], in0=y0, in1=y0, op=OP.mult)
                nc.vector.tensor_tensor(out=t1[:], in0=t1[:], in1=n2[:], op=OP.mult)
                nc.vector.tensor_scalar(out=t1[:], in0=t1[:], scalar1=-0.5, scalar2=1.5,
                                        op0=OP.mult, op1=OP.add)
                nc.vector.tensor_tensor(out=y[:], in0=t1[:], in1=y0, op=OP.mult)
                nc.vector.tensor_tensor(out=t1[:], in0=y[:], in1=y[:], op=OP.mult)
                nc.vector.tensor_tensor(out=t1[:], in0=t1[:], in1=n2[:], op=OP.mult)
                nc.vector.tensor_scalar(out=t1[:], in0=t1[:],
                                        scalar1=-0.5 * cmul, scalar2=1.5 * cmul,
                                        op0=OP.mult, op1=OP.add)
                cs = sc.tile([128, W], outdt, tag=tag + "_c", name="cs")
                nc.vector.tensor_tensor(out=cs[:], in0=t1[:], in1=y[:], op=OP.mult)
                return cs

            def norm_chunk(arr, c0, eps, cmul):
                """cs[:, c0:c0+CH] = cmul / max(|row|, eps) for one CH-tile chunk.
                All-DVE: squares+reduce then bit-trick rsqrt."""
                n2 = sc.tile([128, CH], F32, tag="nrm_n2", name="n2")
                nc.vector.tensor_tensor(out=SQ[:, :CH, :], in0=arr[:, c0:c0 + CH, :],
                                        in1=arr[:, c0:c0 + CH, :], op=OP.mult)
                nc.vector.tensor_reduce(out=n2[:], in_=SQ[:, :CH, :],
                                        axis=AX.X, op=OP.add)
                nc.vector.tensor_scalar_max(out=n2[:], in0=n2[:], scalar1=max(eps * eps, 1e-37))
                return rsqrt_dve(n2, CH, cmul, BF, "nrm")

            # ================= ODE: RK4 =================
            first_ks = [True]

            def tail_chunks(c_stage, rho, last):
                f = float(rho) / float(c_stage)
                first = first_ks[0]
                first_ks[0] = False

                def mk(c0):
                    def run():
                        c1 = c0 + CH
                        cs = norm_chunk(DH, c0, 1e-12, c_stage)
                        nc.vector.tensor_tensor(
                            out=DH[:, c0:c1, :], in0=DH[:, c0:c1, :],
                            in1=cs[:, :, None].to_broadcast([128, CH, 128]),
                            op=OP.mult)
                        if not last:
                            nc.gpsimd.tensor_tensor(out=H[:, c0:c1, :],
                                                    in0=X[:, c0:c1, :],
                                                    in1=DH[:, c0:c1, :], op=OP.add)
                        if first:
                            nc.vector.tensor_scalar_mul(out=KS[:, c0:c1, :],
                                                        in0=DH[:, c0:c1, :], scalar1=f)
                        else:
                            tmp = sc.tile([128, CH, 128], BF, tag="kstmp", name="tmp")
                            nc.vector.tensor_scalar_mul(out=tmp[:], in0=DH[:, c0:c1, :],
                                                        scalar1=f)
                            nc.vector.tensor_tensor(out=KS[:, c0:c1, :],
                                                    in0=KS[:, c0:c1, :], in1=tmp[:],
                                                    op=OP.add)
                    return run
                return [mk(c0) for c0 in range(0, NT, CH)]

            def first_stage_chunks():
                """X normalize + t=0 shortcut dh + first tail, per chunk."""
                tl = tail_chunks(dt2, dt6, False)

                def mk(c0):
                    def run():
                        c1 = c0 + CH
                        cs = norm_chunk(X, c0, 1e-12, 1.0)
                        nc.vector.tensor_tensor(
                            out=X[:, c0:c1, :], in0=X[:, c0:c1, :],
                            in1=cs[:, :, None].to_broadcast([128, CH, 128]),
                            op=OP.mult)
                        nc.vector.tensor_tensor(
                            out=DH[:, c0:c1, :],
                            in0=u0_s[:, None, :].to_broadcast([128, CH, 128]),
                            in1=X[:, c0:c1, :], op=OP.subtract)
                        nc.gpsimd.tensor_tensor(
                            out=DH[:, c0:c1, :], in0=DH[:, c0:c1, :],
                            in1=omz0_s[:, None, :].to_broadcast([128, CH, 128]),
                            op=OP.mult)
                        tl[c0 // CH]()
                    return run
                return [mk(c0) for c0 in range(0, NT, CH)]

            def full_eval(st_res, pre):
                # z-columns of xrz/hrz/b_rz host-negated -> sigmoid gives [r | 1-z].
                # Software-pipelined: stage deps are >=1 step old so each
                # engine queue never head-of-line blocks.
                NS = NT // 2
                agg = [None] * NS; prz = [None] * NS; pu = [None] * NS
                sxt = [None] * NS; ghT = [None] * NS; sig = [None] * NS
                rh = [None] * NS; uT = [None] * NS

                def stA(s):  # PE: aggregation matmuls
                    j = 2 * s
                    a = psA2.tile([128, 512], F32, tag="aggP", space="PSUM")
                    MM(out=a[:, 0:128], lhsT=X[:, j, :], rhs=st_res[:, j, :],
                       start=True, stop=True, skip_group_check=True)
                    MM(out=a[:, 128:256], lhsT=X[:, j + 1, :], rhs=st_res[:, j + 1, :],
                       start=True, stop=True, skip_group_check=True)
                    MM(out=a[:, 256:384], lhsT=H[:, j, :], rhs=st_res[:, j, :],
                       start=True, stop=True, skip_group_check=True)
                    MM(out=a[:, 384:512], lhsT=H[:, j + 1, :], rhs=st_res[:, j + 1, :],
                       start=True, stop=True, skip_group_check=True)
                    agg[s] = a

                def stB(s):  # ACT+DVE: PSUM -> SBUF gate inputs
                    sxt[s] = scD.tile([128, 256], BF, tag="sxt", name="sxt")
                    nc.scalar.copy(out=sxt[s][:], in_=agg[s][:, 0:256])
                    ghT[s] = sc.tile([128, 256], BF, tag="ghT", name="ghT")
                    nc.vector.tensor_copy(out=ghT[s][:], in_=agg[s][:, 256:512])

                def stC(s):  # PE: r/z gate matmuls
                    p = psE.tile([128, 512], F32, tag="przP", space="PSUM")
                    for k in range(2):
                        o = 256 * k
                        MM(out=p[:, o:o + 256], lhsT=ghT[s][:, 128 * k:128 * (k + 1)],
                           rhs=hrz_s[:], start=True, stop=False, skip_group_check=True)
                        MM(out=p[:, o:o + 256], lhsT=sxt[s][:, 128 * k:128 * (k + 1)],
                           rhs=xrz_s[:], start=False, stop=False, skip_group_check=True)
                        MM(out=p[:, o:o + 256], lhsT=ones_s[:], rhs=brz2_s[:, 0:256],
                           start=False, stop=True, skip_group_check=True)
                    prz[s] = p
                    ghT[s] = None

                def stD(s):  # ACT sigmoid
                    sig[s] = scD.tile([128, 4, 128], BF, tag="sig", name="sig")
                    nc.scalar.activation(out=sig[s][:], in_=prz[s][:], func=AF.Sigmoid)
                    prz[s] = None

                def stE(s):  # DVE: r * h
                    j = 2 * s
                    rh[s] = sc.tile([128, 2, 128], BF, tag="rh", name="rh")
                    nc.vector.tensor_tensor(out=rh[s][:], in0=sig[s][:, 0::2, :],
                                            in1=H[:, j:j + 2, :], op=OP.mult)

                def stF(s):  # PE: (r*h) aggregation matmuls
                    j = 2 * s
                    p = psU.tile([128, 512], F32, tag="puP", space="PSUM")
                    MM(out=p[:, 0:128], lhsT=rh[s][:, 0, :], rhs=st_res[:, j, :],
                       start=True, stop=True, skip_group_check=True)
                    MM(out=p[:, 128:256], lhsT=rh[s][:, 1, :], rhs=st_res[:, j + 1, :],
                       start=True, stop=True, skip_group_check=True)
                    pu[s] = p
                    rh[s] = None

                def stG(s):  # ACT: PSUM -> SBUF for u-gate lhsT
                    uT[s] = sc.tile([128, 256], BF, tag="uT", name="uT")
                    nc.scalar.copy(out=uT[s][:], in_=pu[s][:, 0:256])

                def stH(s):  # PE: u gate matmuls
                    for k in range(2):
                        o = 256 + 128 * k
                        MM(out=pu[s][:, o:o + 128], lhsT=uT[s][:, 128 * k:128 * (k + 1)],
                           rhs=hh_s[:], start=True, stop=False, skip_group_check=True)
                        MM(out=pu[s][:, o:o + 128], lhsT=sxt[s][:, 128 * k:128 * (k + 1)],
                           rhs=xh_s[:], start=False, stop=False, skip_group_check=True)
                        MM(out=pu[s][:, o:o + 128], lhsT=ones_s[:], rhs=bu2_s[:, 0:128],
                           start=False, stop=True, skip_group_check=True)
                    uT[s] = None
                    sxt[s] = None

                def stI(s):  # ACT tanh + Pool epilogue -> DH
                    j = 2 * s
                    uP = sc.tile([128, 2, 128], BF, tag="ut")
                    nc.scalar.activation(out=uP[:], in_=pu[s][:, 256:512], func=AF.Tanh)
                    nc.gpsimd.tensor_tensor(out=uP[:], in0=uP[:], in1=H[:, j:j + 2, :],
                                            op=OP.subtract)
                    nc.gpsimd.tensor_tensor(out=DH[:, j:j + 2, :], in0=uP[:],
                                            in1=sig[s][:, 1::2, :], op=OP.mult)
                    pu[s] = None
                    sig[s] = None
                    agg[s] = None

                for s in range(NS + 3):
                    if s % (CH // 2) == 0 and s // (CH // 2) < len(pre):
                        pre[s // (CH // 2)]()
                    if s < NS:
                        stA(s)
                    if s >= 1 and s - 1 < NS:
                        stC(s - 1)
                    if s >= 2 and s - 2 < NS:
                        stF(s - 2)
                    if s >= 3 and s - 3 < NS:
                        stH(s - 3)
                    if s < NS:
                        stB(s)
                    if s >= 1 and s - 1 < NS:
                        stD(s - 1)
                        stE(s - 1)
                    if s >= 2 and s - 2 < NS:
                        stG(s - 2)
                    if s >= 3 and s - 3 < NS:
                        stI(s - 3)

            if has_t0:
                # ST0 path needs H initialized to X (f(0, feat) uses h=feat)
                def init_chunks():
                    def mk(c0):
                        def run():
                            c1 = c0 + CH
                            cs = norm_chunk(X, c0, 1e-12, 1.0)
                            nc.vector.tensor_tensor(
                                out=X[:, c0:c1, :], in0=X[:, c0:c1, :],
                                in1=cs[:, :, None].to_broadcast([128, CH, 128]),
                                op=OP.mult)
                            nc.vector.tensor_copy(out=H[:, c0:c1, :], in_=X[:, c0:c1, :])
                        return run
                    return [mk(c0) for c0 in range(0, NT, CH)]
                full_eval(ST0, init_chunks())
                full_eval(STH, tail_chunks(dt2, dt6, False))
            else:
                full_eval(STH, first_stage_chunks())
            full_eval(STH, tail_chunks(dt2, 2.0 * dt6, False))
            full_eval(STF, tail_chunks(float(dt_val), 2.0 * dt6, False))
            # final tail + H = normalize(X + KS), interleaved with readout
            # transposes chunk by chunk
            final_tail = tail_chunks(1.0, dt6, True)
            XT = state.tile([128, NT, 128], BF, tag="X")  # X dead after H formed
            flT = per.tile([128, 128], BF, tag="flTs")
            for c0 in range(0, NT, CH):
                c1 = c0 + CH
                final_tail[c0 // CH]()
                nc.vector.tensor_tensor(out=H[:, c0:c1, :], in0=X[:, c0:c1, :],
                                        in1=KS[:, c0:c1, :], op=OP.add)
                cs = norm_chunk(H, c0, 1e-30, 1.0)
                nc.vector.tensor_tensor(out=H[:, c0:c1, :], in0=H[:, c0:c1, :],
                                        in1=cs[:, :, None].to_broadcast([128, CH, 128]),
                                        op=OP.mult)
                for j in range(c0, c1, 2):
                    xtP = psU.tile([128, 256], BF, tag="puP", space="PSUM")
                    nc.tensor.transpose(out=xtP[:, 0:128], in_=H[:, j, :], identity=id_s[:])
                    nc.tensor.transpose(out=xtP[:, 128:256], in_=H[:, j + 1, :], identity=id_s[:])
                    if (j // 2) % 2 == 0:
                        nc.scalar.copy(out=XT[:, j:j + 2, :], in_=xtP[:])
                    else:
                        nc.vector.tensor_copy(out=XT[:, j:j + 2, :], in_=xtP[:])
                    nc.vector.tensor_copy(out=flT[:, j * SPT:(j + 2) * SPT],
                                          in_=XT[:, j:j + 2, c.P - 1::PADP])
            # fvS[sess, d] = H_last @ fc_vw + fc_vb   (sessions on partitions)
            pfv = psA2.tile([128, 512], F32, tag="aggP", space="PSUM")
            nc.tensor.matmul(out=pfv[:, 0:128], lhsT=flT[:], rhs=fcvw_s[:],
                             start=True, stop=False, skip_group_check=True)
            nc.tensor.matmul(out=pfv[:, 0:128], lhsT=ones_s[:], rhs=bvb_s[:],
                             start=False, stop=True, skip_group_check=True)
            fvS = per.tile([128, 128], BF, tag="fvS")
            nc.scalar.copy(out=fvS[:], in_=pfv[:, 0:128])

            ee = per.tile([128, NT], BF, tag="ee")
            ecolF = per.tile([128, NT], F32, tag="ecolF")
            for j in range(0, NT, 2):
                peP = psA2.tile([128, 512], F32, tag="aggP", space="PSUM")
                for k in range(2):
                    o = 128 * k
                    s0 = (j + k) * SPT
                    MM(out=peP[:, o:o + 128], lhsT=XT[:, j + k, :], rhs=fcu_s[:],
                       start=True, stop=False, skip_group_check=True)
                    # selection matmul: one-hot [sess, node] map adds fv[sess(n), :]
                    MM(out=peP[:, o:o + 128], lhsT=sel_s[:, j + k, :], rhs=fvS[:],
                       start=False, stop=True, skip_group_check=True)
                sg = sc.tile([128, 2, 128], BF, tag="sg")
                nc.scalar.activation(out=sg[:], in_=peP[:, 0:256], func=AF.Sigmoid)
                nc.vector.tensor_tensor(out=sg[:], in0=sg[:],
                                        in1=fce_s[:, None, :].to_broadcast([128, 2, 128]),
                                        op=OP.mult)
                nc.vector.tensor_reduce(out=ecolF[:, j:j + 2], in_=sg[:], axis=AX.X, op=OP.add)
            nc.scalar.activation(out=ee[:], in_=ecolF[:], func=AF.Exp)
            ssum_ps = psE.tile([SPT, NT], F32, tag="przP", space="PSUM")
            nc.tensor.matmul(out=ssum_ps[:], lhsT=ptf_s[:], rhs=ee[:], start=True, stop=True)
            rsum = per.tile([SPT, NT], F32, tag="rsum")
            nc.vector.reciprocal(out=rsum[:], in_=ssum_ps[:])
            rsumb = per.tile([SPT, NT], BF, tag="rsumb")
            nc.vector.tensor_copy(out=rsumb[:], in_=rsum[:])
            sb_ps = psE.tile([128, NT], F32, tag="przP", space="PSUM")
            nc.tensor.matmul(out=sb_ps[:], lhsT=pt2_s[:], rhs=rsumb[:], start=True, stop=True)
            alpha = per.tile([128, NT], BF, tag="alpha")
            nc.vector.tensor_tensor(out=alpha[:], in0=ee[:], in1=sb_ps[:], op=OP.mult)

            srg_ps = psG.tile([128, 128], F32, tag="pSRG", space="PSUM")
            aptA = per.tile([128, NT, SPT], BF, tag="aptA")
            nc.vector.tensor_tensor(out=aptA[:],
                                    in0=ptf_s[:, None, :].to_broadcast([128, NT, SPT]),
                                    in1=alpha[:, :, None].to_broadcast([128, NT, SPT]),
                                    op=OP.mult)
            for j in range(NT):
                s0 = j * SPT
                nc.tensor.matmul(out=srg_ps[:, s0:s0 + SPT], lhsT=H[:, j, :], rhs=aptA[:, j, :],
                                 start=True, stop=True, skip_group_check=True)
            srgT = per.tile([128, 128], BF, tag="srgT")
            nc.vector.tensor_copy(out=srgT[:], in_=srg_ps[:])
            psr = psE.tile([128, 512], F32, tag="przP", space="PSUM")
            nc.tensor.matmul(out=psr[:, 0:128], lhsT=flT[:], rhs=fsra_s[:],
                             start=True, stop=False, skip_group_check=True)
            nc.tensor.matmul(out=psr[:, 0:128], lhsT=srgT[:], rhs=fsrb_s[:],
                             start=False, stop=True, skip_group_check=True)
            sr = per.tile([128, 128], BF, tag="sr")
            n2s = sc.tile([128, 1], F32, tag="srn2")
            sq1 = sc.tile([128, 128], F32, tag="srsq")
            nc.scalar.activation(out=sq1[:], in_=psr[:, 0:128], func=AF.Square, accum_out=n2s[:])
            nc.vector.tensor_scalar_max(out=n2s[:], in0=n2s[:], scalar1=1e-24)
            recs = rsqrt_dve(n2s, 1, 1.0, F32, "srr")
            nc.vector.tensor_scalar(out=sr[:], in0=psr[:, 0:128], scalar1=recs[:],
                                    scalar2=None, op0=OP.mult)
            srT_ps = psU.tile([128, 256], BF, tag="puP", space="PSUM")
            nc.tensor.transpose(out=srT_ps[:, 0:128], in_=sr[:], identity=id_s[:])
            srT = per.tile([128, 128], BF, tag="srTs")
            nc.vector.tensor_copy(out=srT[:], in_=srT_ps[:, 0:128])

            # ========== logits + log_softmax (own 128 sessions, full vocab) ==========
            # tgt streamed from DRAM once. Chunks alternate storage format in
            # SBUF (state pool space, released here): even chunks keep
            # exp(12L) -> pass 2 = Ln(LOG * 1/Z) on ACT; odd chunks keep raw
            # 12L (DVE cast) -> pass 2 = +(-lnZ) on DVE. Splits pass-2 work
            # across both engines.
            state.release()
            logp = tc.alloc_tile_pool(name="logp", bufs=1)

            def pl_tile(ch):
                if ch % 3 == 0:
                    plt = psE.tile([128, 512], F32, tag="przP", space="PSUM")
                elif ch % 3 == 1:
                    plt = psU.tile([128, 512], F32, tag="puP", space="PSUM")
                else:
                    plt = psA2.tile([128, 512], F32, tag="aggP", space="PSUM")
                return plt

            NCHUNK = (V + 511) // 512
            LOG = logp.tile([128, NCHUNK * 512], BF, tag="LOG")
            separt = per.tile([128, NCHUNK], F32, tag="separt")
            ch = 0
            for t0 in range(0, V, TSC):
                tw = min(TSC, V - t0)
                tg = strm.tile([128, TSC], BF, tag="tgstream")
                nc.sync.dma_start(out=tg[:, :tw], in_=tgtT[:, t0:t0 + tw])
                for q0 in range(0, tw, 512):
                    cw = min(512, tw - q0)
                    pl = pl_tile(ch)
                    MM(out=pl[:, :cw], lhsT=srT[:], rhs=tg[:, q0:q0 + cw],
                       start=True, stop=True)
                    lg = LOG[:, ch * 512:ch * 512 + cw]
                    if ch % 4 == 0:
                        nc.scalar.activation(out=lg, in_=pl[:, :cw], func=AF.Exp,
                                             scale=SCALE, accum_out=separt[:, ch:ch + 1])
                    else:
                        nc.vector.tensor_scalar_mul(out=lg, in0=pl[:, :cw], scalar1=SCALE)
                        escr = ob.tile([128, 512], BF, tag="escr")
                        nc.scalar.activation(out=escr[:, :cw], in_=pl[:, :cw], func=AF.Exp,
                                             scale=SCALE, accum_out=separt[:, ch:ch + 1])
                    ch += 1
            sumexp = per.tile([128, 1], F32, tag="sumexp")
            nc.vector.tensor_reduce(out=sumexp[:], in_=separt[:], axis=AX.X, op=OP.add)
            recz = per.tile([128, 1], F32, tag="recz")
            nc.vector.reciprocal(out=recz[:], in_=sumexp[:])
            nlog = per.tile([128, 1], F32, tag="nlog")
            nc.scalar.activation(out=nlog[:], in_=sumexp[:], func=AF.Ln)
            nc.vector.tensor_scalar_mul(out=nlog[:], in0=nlog[:], scalar1=-1.0)

            OBW = 4096  # output block: 8 chunks, ACT/DVE alternating
            for b0 in range(0, V, OBW):
                bw = min(OBW, V - b0)
                outb = strm.tile([128, OBW], BF, tag="lslB")
                for q0 in range(0, bw, 512):
                    cw = min(512, bw - q0)
                    chq = (b0 + q0) // 512
                    if chq % 4 == 0:
                        nc.scalar.activation(out=outb[:, q0:q0 + cw],
                                             in_=LOG[:, b0 + q0:b0 + q0 + cw],
                                             func=AF.Ln, scale=recz[:])
                    else:
                        nc.vector.tensor_scalar_add(out=outb[:, q0:q0 + cw],
                                                    in0=LOG[:, b0 + q0:b0 + q0 + cw],
                                                    scalar1=nlog[:])
                nc.sync.dma_start(out=out_slice[:, b0:b0 + bw], in_=outb[:, :bw])
            logp.release()

    nc.compile()
    return nc


# ====================== host preprocessing =========================

def prep_inputs(cfg, inputs):
    c = cfg
    V, B, P, NC, PADP = c.V, c.B, c.P, c.NC, c.PADP
    NT, SPT, SC = c.NT, c.SPT, c.SC
    f32 = np.float32

    iid = np.asarray(inputs["iid"]).astype(np.int64)
    esrc = np.asarray(inputs["edge_src"]).astype(np.int64)
    edst = np.asarray(inputs["edge_dst"]).astype(np.int64)
    ew = np.asarray(inputs["edge_w"]).astype(f32)
    et = np.asarray(inputs["edge_t"]).astype(f32)
    emb = np.ascontiguousarray(np.asarray(inputs["embedding"]).astype(f32))
    last_nodes = np.asarray(inputs["last_nodes"]).astype(np.int64)
    assert np.array_equal(last_nodes, np.arange(B) * P + (P - 1)), "unexpected last_nodes"
    es_sess = esrc // P
    assert np.array_equal(es_sess, edst // P), "edges cross sessions"

    dt = float(et.max())
    has_t0 = bool((et <= 0.0).any())

    g = lambda k: np.asarray(inputs[k], f32)
    z0 = 1.0 / (1.0 + np.exp(-(g("bxz") + g("bhz")).astype(np.float64)))
    u0 = np.tanh((g("bxh") + g("bhh")).astype(np.float64))
    omz0 = (1.0 - z0).astype(f32)
    u0 = u0.astype(f32)

    ls = (esrc % P).astype(np.int64)
    ld_ = (edst % P).astype(np.int64)
    no_self = esrc != edst

    Mw = np.zeros((B, PADP, PADP), f32)
    np.add.at(Mw, (es_sess, ls, ld_), ew)
    ws_in = Mw.sum(axis=1)
    ws_out = Mw.sum(axis=2)
    M1T = Mw / np.where(ws_in > 0, ws_in, 1.0)[:, None, :]
    M2T = (Mw / np.where(ws_out > 0, ws_out, 1.0)[:, :, None]).transpose(0, 2, 1)

    def sym_norm(mask):
        Mm = np.zeros((B, PADP, PADP), f32)
        np.add.at(Mm, (es_sess, ls, ld_), mask.astype(f32))
        S = Mm + Mm.transpose(0, 2, 1)
        deg = S.sum(axis=2)
        nrm = np.maximum(deg, 1.0) ** -0.5
        return (nrm[:, :, None] * S * nrm[:, None, :]).astype(f32)

    St_h = sym_norm((et <= np.float32(dt * 0.5)) & no_self)
    St_f = sym_norm((et <= np.float32(dt)) & no_self)
    St_0 = sym_norm((et <= np.float32(0.0)) & no_self) if has_t0 else None

    def blocks_to_tilesT(Bm, core, width=128):
        out = np.zeros((NT, 128, width), f32)
        for s in range(SC):
            j, k = s // SPT, s % SPT
            out[j, k * PADP:(k + 1) * PADP, k * PADP:(k + 1) * PADP] = Bm[core * SC + s]
        return np.ascontiguousarray(out.transpose(1, 0, 2).astype(BF_NP))

    # host-side embedding gather + normalize (input sharding prep)
    feat = emb[iid]
    feat = feat / (np.linalg.norm(feat, axis=1, keepdims=True) + 1e-12)
    featp = np.zeros((B, PADP, 128), f32)
    featp[:, :P, :] = feat.reshape(B, P, 128)
    featp = featp.reshape(NC, SC // SPT, SPT * PADP, 128)  # [NC, NT, 128, 128]

    # normalized target, transposed (full vocab, shared by all cores)
    tgt = emb / (np.linalg.norm(emb, axis=1, keepdims=True) + 1e-12)
    tgtT_full = np.ascontiguousarray(tgt.T.astype(BF_NP))  # [128, V]

    W1, W2 = g("W1"), g("W2")
    gwih, gwhh = g("gru_wih"), g("gru_whh")
    gbih, gbhh = g("gru_bih"), g("gru_bhh")
    P1 = (W1 @ gwih.T[0:256, :]).astype(f32)
    P2 = (W2 @ gwih.T[256:512, :]).astype(f32)
    whhT = np.ascontiguousarray(gwhh.T).copy()
    b_pg = gbih.copy()
    b_pg[0:256] += gbhh[0:256]
    b_h3 = gbhh[256:384].copy()
    # negate z columns so sigmoid(pg[0:256]) = [r | 1-z]
    P1[:, 128:256] *= -1.0
    P2[:, 128:256] *= -1.0
    whhT[:, 128:256] *= -1.0
    b_pg[128:256] *= -1.0

    Wxrz = np.concatenate([g("Wxr"), g("Wxz")], axis=1)
    Whrz = np.concatenate([g("Whr"), g("Whz")], axis=1)
    b_rz = np.concatenate([g("bxr") + g("bhr"), g("bxz") + g("bhz")])
    b_u = g("bxh") + g("bhh")
    # negate z columns -> sigmoid(prz) = [r | 1-z]
    Wxrz[:, 128:256] *= -1.0
    Whrz[:, 128:256] *= -1.0
    b_rz[128:256] *= -1.0

    ptf = np.zeros((128, SPT), f32)
    pt2 = np.zeros((SPT, 128), f32)
    for p in range(128):
        j = p // PADP
        pt2[j, p] = 1.0
        if p % PADP < P:
            ptf[p, j] = 1.0
    # selh[s, T, n] = 1 iff local session s == T*SPT + n//PADP
    selh = np.zeros((128, NT, 128), f32)
    for s in range(128):
        selh[s, s // SPT, (s % SPT) * PADP:(s % SPT + 1) * PADP] = 1.0

    bf = lambda a: np.ascontiguousarray(np.asarray(a, f32).astype(BF_NP))
    shared = dict(
        w_p1=bf(P1), w_p2=bf(P2), w_whhT=bf(whhT),
        w_xrz=bf(Wxrz), w_xh=bf(g("Wxh")), w_hrz=bf(Whrz), w_hh=bf(g("Whh")),
        w_fcu=bf(g("fc_u")), w_fcvw=bf(g("fc_vw")),
        w_fsra=bf(g("fc_sr")[0:128, :]), w_fsrb=bf(g("fc_sr")[128:256, :]),
        b_pgx=bf(np.concatenate([b_pg, b_h3])[None, :]),
        bpg_rep=bf(np.repeat(np.concatenate([b_pg, b_h3])[None, :], 128, axis=0)),
        brz_rep=bf(np.repeat(np.tile(b_rz, 2)[None, :], 128, axis=0)),
        b_rz2=bf(np.tile(b_rz, 2)[None, :]),
        b_u2=bf(np.tile(b_u, 2)[None, :]),
        bvb_row=bf(g("fc_vb")[None, :]),
        ones1=bf(np.ones((1, 128), f32)),
        ptf=bf(ptf), pt2=bf(pt2), sel=bf(selh),
        fce_rep=bf(np.repeat(g("fc_e")[None, :], 128, axis=0)),
        omz0_rep=bf(np.repeat(omz0[None, :], 128, axis=0)),
        u0_rep=bf(np.repeat(u0[None, :], 128, axis=0)),
        identity=bf(np.eye(128, dtype=f32)),
        tgtT=tgtT_full,
    )

    in_maps = []
    for core in range(NC):
        m = dict(shared)
        m["x0"] = np.ascontiguousarray(
            featp[core].transpose(1, 0, 2).astype(BF_NP))  # [128, NT, 128]
        m["x0T"] = np.ascontiguousarray(
            featp[core].transpose(2, 0, 1).astype(BF_NP))  # per-tile transpose
        m["m12tT"] = np.ascontiguousarray(np.concatenate(
            [blocks_to_tilesT(M1T, core), blocks_to_tilesT(M2T, core)], axis=2))
        m["sthT"] = blocks_to_tilesT(St_h, core)
        m["stfT"] = blocks_to_tilesT(St_f, core)
        if has_t0:
            m["st0T"] = blocks_to_tilesT(St_0, core)
        in_maps.append(m)
    return in_maps, dt, has_t0


_NC_CACHE = {}


def kernel(**inputs):
    cfg = FULL
    in_maps, dt, has_t0 = prep_inputs(cfg, inputs)
    key = (round(dt, 9), has_t0)
    if key not in _NC_CACHE:
        _NC_CACHE[key] = build_nc(cfg, dt, has_t0, cfg.NC)
    nc = _NC_CACHE[key]
    res = run_bass_kernel_spmd(nc, in_maps, core_ids=list(range(cfg.NC)),
                               trace=bool(int(os.environ.get("KTRACE", "0"))))
    kernel.last_result = res
    return np.concatenate(
        [np.asarray(res.results[c]["out_slice"]).astype(np.float32)
         for c in range(cfg.NC)], axis=0)
